# revision 3
# baseline (speedup 1.0000x reference)
"""3-layer GCN forward (GCNConv x3 + log_softmax) on 8 Trainium2 cores.

v2: batched dma_gather edition.
  Per layer: tiled GEMM (fp32) + dinv_src row scale -> local Z block cast to
  bf16; AllGather Z (bf16) across the 8 cores into a shared DRAM replica;
  aggregation via InstDMAGatherAnt: the zf replica is split into 4 tables of
  25088 rows (int16 index limit), each dma_gather instruction fetches
  KCOL*128 = 2048 rows (vs 128 for the old per-column indirect DMA, whose
  ~1us/instr SWDGE fixed cost dominated the baseline at 5.5ms of gpsimd
  time). Gathered bf16 rows are accumulated on the tensor engine via
  identity-matmul into PSUM (bf16 = full PE rate), then dinv_dst scale +
  bias + relu (or log_softmax on the last layer).

Grid: 98 groups of 128 output rows per core; per (group, table) a
rectangular [128 x d_qg] slot grid, padded slots gather a zero row.
Self-loops are kept in the grid. Node permutation deals degree-ranked nodes
round-robin across cores so all cores' group degree profiles align.
"""
import numpy as np

NCORES = 8
N = 100000
NBLK = 12500
NPAD = 12544            # 98 * 128
NGRP = NPAD // 128      # 98
C = 128
COUT = 47
NQ = 4                  # source tables (int16 gather-index limit)
TBL = 2 * NPAD          # 25088 rows per table = 2 core blocks
ZROW = NBLK             # a zero pad row local to each table
KCOL = 16               # gather columns per dma_gather instruction


def _preprocess(x, edge_index, W1, b1, W2, b2, W3, b3):
    x = np.asarray(x, np.float32)
    ei = np.asarray(edge_index)
    loop = np.arange(N, dtype=np.int64)
    src = np.concatenate([ei[0], loop]).astype(np.int64)
    dst = np.concatenate([ei[1], loop]).astype(np.int64)

    deg = np.bincount(dst, minlength=N).astype(np.float32)
    dinv = 1.0 / np.sqrt(np.maximum(deg, 1.0))

    # deal degree-ranked nodes round-robin across cores so all 8 cores'
    # group degree profiles align (minimizes cross-core max padding)
    rank = np.argsort(-deg, kind="stable")
    perm = np.empty(N, np.int64)
    for k in range(NCORES):
        perm[k * NBLK:(k + 1) * NBLK] = rank[k::NCORES]
    inv = np.empty(N, np.int64)
    inv[perm] = np.arange(N)

    srcp = inv[src]
    dstp = inv[dst]
    ksrc = srcp // NBLK
    srcg = ksrc * NPAD + (srcp - ksrc * NBLK)     # padded-global coords
    q_e = srcg // TBL                              # source table
    lidx = srcg - q_e * TBL                        # local row in table
    kdst = dstp // NBLK
    rloc = dstp - kdst * NBLK

    dinv_p = dinv[perm]

    # sort edges by (dst core, table, dst row)
    key = (kdst * NQ + q_e) * NBLK + rloc
    order = np.argsort(key, kind="stable")
    key_s = key[order]
    lidx_s = lidx[order]
    cnt = np.bincount(key_s, minlength=NCORES * NQ * NBLK)
    starts = np.zeros(NCORES * NQ * NBLK + 1, np.int64)
    np.cumsum(cnt, out=starts[1:])
    pos = np.arange(len(order)) - starts[key_s]

    cnt_pad = np.zeros((NCORES, NQ, NPAD), np.int64)
    cnt_pad[:, :, :NBLK] = cnt.reshape(NCORES, NQ, NBLK)
    d_qg = cnt_pad.reshape(NCORES, NQ, NGRP, 128).max(axis=(0, 3))  # [NQ,NGRP]

    off = np.zeros((NQ, NGRP), np.int64)
    ncol = np.zeros(NQ, np.int64)
    for q in range(NQ):
        cs = np.cumsum(d_qg[q])
        off[q, 1:] = cs[:-1]
        ncol[q] = cs[-1]
    ncol_pad = ((ncol + KCOL - 1) // KCOL) * KCOL

    kq_s = key_s // (NQ * NBLK)
    q_s = (key_s // NBLK) % NQ
    rloc_s = key_s % NBLK
    g_s = rloc_s // 128
    p_s = rloc_s % 128
    col_s = off[q_s, g_s] + pos

    tabs = [np.full((NCORES, 128, int(ncol_pad[q])), ZROW, np.int16)
            for q in range(NQ)]
    for q in range(NQ):
        m = q_s == q
        tabs[q][kq_s[m], p_s[m], col_s[m]] = lidx_s[m].astype(np.int16)

    # wrapped int16 layout for dma_gather: slot i=(col*128+pp) lives at
    # sbuf[p, col*8 + pp//16] for p%16 == pp%16, replicated across bands
    W_TOT = int(ncol_pad.sum())
    soff = np.zeros(NQ + 1, np.int64)
    np.cumsum(ncol_pad, out=soff[1:])
    gidxw = np.zeros((NCORES, 128, 8 * W_TOT), np.int16)
    for k in range(NCORES):
        for q in range(NQ):
            a = tabs[q][k]                                 # [128, ncol]
            t = a.reshape(8, 16, -1).transpose(1, 2, 0)    # [16, ncol, 8]
            w = np.ascontiguousarray(t).reshape(16, -1)    # [16, ncol*8]
            gidxw[k][:, soff[q] * 8:soff[q + 1] * 8] = np.tile(w, (8, 1))

    dinv_loc = np.zeros((NCORES, 128, NGRP), np.float32)
    dv = dinv_p.reshape(NCORES, NBLK)
    for k in range(NCORES):
        full = np.zeros(NPAD, np.float32)
        full[:NBLK] = dv[k]
        dinv_loc[k] = full.reshape(NGRP, 128).T

    xp = x[perm]
    xblk = np.zeros((NCORES, NPAD, C), np.float32)
    for k in range(NCORES):
        xblk[k, :NBLK] = xp[k * NBLK:(k + 1) * NBLK]

    Ws = [np.ascontiguousarray(W, np.float32) for W in (W1, W2, W3)]
    bb = [np.tile(np.asarray(b, np.float32)[None, :], (128, 1))
          for b in (b1, b2, b3)]

    in_maps = []
    for k in range(NCORES):
        in_maps.append({
            "xblk": np.ascontiguousarray(xblk[k]),
            "gidx": np.ascontiguousarray(gidxw[k]),
            "dinv": np.ascontiguousarray(dinv_loc[k]),
            "w1": Ws[0], "w2": Ws[1], "w3": Ws[2],
            "bb1": np.ascontiguousarray(bb[0]),
            "bb2": np.ascontiguousarray(bb[1]),
            "bb3": np.ascontiguousarray(bb[2]),
        })
    return in_maps, d_qg, off, ncol_pad, perm


def _build(d_qg, off, ncol_pad):
    from concourse import bacc, bass, mybir, tile
    from concourse.masks import make_identity
    from concourse.library_config import mlp
    f32 = mybir.dt.float32
    bf16 = mybir.dt.bfloat16
    i16 = mybir.dt.int16
    couts = [C, C, COUT]
    soff = np.zeros(NQ + 1, np.int64)
    np.cumsum(ncol_pad, out=soff[1:])
    W_TOT = int(ncol_pad.sum())

    nc = bacc.Bacc("TRN2", target_bir_lowering=False, debug=False,
                   num_devices=NCORES)
    xblk = nc.dram_tensor("xblk", [NPAD, C], f32, kind="ExternalInput")
    gidx = nc.dram_tensor("gidx", [128, 8 * W_TOT], i16, kind="ExternalInput")
    dinv = nc.dram_tensor("dinv", [128, NGRP], f32, kind="ExternalInput")
    w_in = [nc.dram_tensor(f"w{l+1}", [C, couts[l]], f32,
                           kind="ExternalInput") for l in range(3)]
    bb_in = [nc.dram_tensor(f"bb{l+1}", [128, couts[l]], f32,
                            kind="ExternalInput") for l in range(3)]
    out_d = nc.dram_tensor("out", [NPAD, COUT], f32, kind="ExternalOutput")

    zs = [nc.dram_tensor(f"zs{l}", [NPAD, C], bf16) for l in range(3)]
    zf = [nc.dram_tensor(f"zf{l}", [NCORES * NPAD, C], bf16,
                         addr_space="Shared") for l in range(3)]

    with tile.TileContext(nc) as tc:
        with tc.tile_pool(name="const", bufs=1) as cpool, \
             tc.tile_pool(name="hbuf", bufs=1) as hpool, \
             tc.tile_pool(name="gath", bufs=8) as gpool, \
             tc.tile_pool(name="work", bufs=4) as wpool, \
             tc.tile_pool(name="ps_t", bufs=2, space="PSUM") as ps_t, \
             tc.tile_pool(name="ps_z", bufs=2, space="PSUM") as ps_z, \
             tc.tile_pool(name="ps_g", bufs=2, space="PSUM") as ps_g:

            nc.gpsimd.load_library(mlp)
            ident = cpool.tile([128, 128], f32)
            make_identity(nc, ident[:])
            identB = cpool.tile([128, 128], bf16, name="identB")
            make_identity(nc, identB[:])
            idx_sb = cpool.tile([128, 8 * W_TOT], i16)
            nc.sync.dma_start(out=idx_sb[:], in_=gidx[:])
            dinv_sb = cpool.tile([128, NGRP], f32)
            nc.sync.dma_start(out=dinv_sb[:], in_=dinv[:])
            w_sb, bb_sb = [], []
            for l in range(3):
                w = cpool.tile([128, couts[l]], f32, name=f"w_sb{l}")
                nc.sync.dma_start(out=w[:], in_=w_in[l][:])
                w_sb.append(w)
                b = cpool.tile([128, couts[l]], f32, name=f"bb_sb{l}")
                nc.sync.dma_start(out=b[:], in_=bb_in[l][:])
                bb_sb.append(b)

            H = hpool.tile([128, NGRP * C], f32)

            def phase_a(lay, g):
                co = couts[lay]
                if lay == 0:
                    hin = wpool.tile([128, C], f32, name="hin")
                    nc.sync.dma_start(
                        out=hin[:], in_=xblk[g * 128:(g + 1) * 128, :])
                    hsrc = hin[:]
                else:
                    hsrc = H[:, g * C:(g + 1) * C]
                pst = ps_t.tile([128, 128], f32, name="pst")
                nc.tensor.transpose(out=pst[:], in_=hsrc, identity=ident[:])
                ht = wpool.tile([128, 128], f32, name="ht")
                nc.vector.tensor_copy(out=ht[:], in_=pst[:])
                psz = ps_z.tile([128, co], f32, name="psz")
                nc.tensor.matmul(out=psz[:], lhsT=ht[:], rhs=w_sb[lay][:],
                                 start=True, stop=True)
                ztb = wpool.tile([128, C], bf16, name="ztb")
                if co < C:
                    nc.vector.memset(ztb[:, co:], 0.0)
                nc.vector.tensor_scalar_mul(out=ztb[:, :co], in0=psz[:],
                                            scalar1=dinv_sb[:, g:g + 1])
                nc.sync.dma_start(out=zs[lay][g * 128:(g + 1) * 128, :],
                                  in_=ztb[:])

            def postproc(lay, g, psg, nqm):
                co = couts[lay]
                tmp = wpool.tile([128, C], f32, name="tmp")
                nc.vector.tensor_copy(out=tmp[:, :co], in_=psg[:, :co])
                for qq in range(1, nqm):
                    nc.vector.tensor_add(out=tmp[:, :co], in0=tmp[:, :co],
                                         in1=psg[:, qq * C:qq * C + co])
                nc.vector.tensor_scalar_mul(out=tmp[:, :co], in0=tmp[:, :co],
                                            scalar1=dinv_sb[:, g:g + 1])
                nc.vector.tensor_add(out=tmp[:, :co], in0=tmp[:, :co],
                                     in1=bb_sb[lay][:])
                if lay < 2:
                    nc.vector.tensor_scalar_max(
                        out=H[:, g * C:(g + 1) * C], in0=tmp[:, :co],
                        scalar1=0.0)
                else:
                    mx = wpool.tile([128, 1], f32, name="mx")
                    nc.vector.tensor_reduce(
                        out=mx[:], in_=tmp[:, :co],
                        axis=mybir.AxisListType.X, op=mybir.AluOpType.max)
                    nmx = wpool.tile([128, 1], f32, name="nmx")
                    nc.vector.tensor_scalar_mul(out=nmx[:], in0=mx[:],
                                                scalar1=-1.0)
                    ex = wpool.tile([128, C], f32, name="ex")
                    ssum = wpool.tile([128, 1], f32, name="ssum")
                    nc.scalar.activation(
                        out=ex[:, :co], in_=tmp[:, :co],
                        func=mybir.ActivationFunctionType.Exp,
                        bias=nmx[:], scale=1.0, accum_out=ssum[:])
                    lse = wpool.tile([128, 1], f32, name="lse")
                    nc.scalar.activation(
                        out=lse[:], in_=ssum[:],
                        func=mybir.ActivationFunctionType.Ln)
                    tot = wpool.tile([128, 1], f32, name="tot")
                    nc.vector.tensor_add(out=tot[:], in0=lse[:], in1=mx[:])
                    ot = wpool.tile([128, COUT], f32, name="ot")
                    nc.vector.tensor_scalar_sub(out=ot[:], in0=tmp[:, :co],
                                                scalar1=tot[:])
                    nc.sync.dma_start(
                        out=out_d[g * 128:(g + 1) * 128, :], in_=ot[:])

            for lay in range(3):
                for g in range(NGRP):
                    phase_a(lay, g)
                nc.gpsimd.collective_compute(
                    "AllGather", mybir.AluOpType.bypass,
                    replica_groups=[list(range(NCORES))],
                    ins=[zs[lay][:, :]], outs=[zf[lay][:, :]])

                issued = [0] * NQ
                tile_of = {}

                def ensure(q, cols_needed):
                    while issued[q] * KCOL < cols_needed:
                        t = issued[q]
                        gs = gpool.tile([128, KCOL, C], bf16, name="gs")
                        base = int(soff[q] + t * KCOL) * 8
                        nc.gpsimd.dma_gather(
                            gs[:], zf[lay][q * TBL:(q + 1) * TBL, :],
                            idx_sb[:, base:base + KCOL * 8],
                            KCOL * 128, KCOL * 128, C,
                            single_packet=False)
                        tile_of[(q, t)] = gs
                        issued[q] += 1

                for g in range(NGRP):
                    chunks = []
                    for q in range(NQ):
                        a = int(off[q][g])
                        b = a + int(d_qg[q][g])
                        ensure(q, b)
                        c0 = a
                        while c0 < b:
                            t = c0 // KCOL
                            run_end = min(b, (t + 1) * KCOL)
                            w = min(4, run_end - c0)
                            chunks.append((q, t, c0 - t * KCOL, w))
                            c0 += w
                    chunks.sort(key=lambda ch: -ch[3])
                    nqm = chunks[0][3]
                    psg = ps_g.tile([128, 4 * C], f32, name="psg")
                    for i, (q, t, o, w) in enumerate(chunks):
                        gsv = tile_of[(q, t)][:].rearrange("p a b -> p (a b)")
                        nc.tensor.matmul(out=psg[:, :w * C], lhsT=identB[:],
                                         rhs=gsv[:, o * C:(o + w) * C],
                                         start=(i == 0),
                                         stop=(i == len(chunks) - 1))
                    postproc(lay, g, psg, nqm)

    nc.compile()
    return nc


def kernel(x, edge_index, W1, b1, W2, b2, W3, b3):
    from concourse.bass_utils import run_bass_kernel_spmd

    in_maps, d_qg, off, ncol_pad, perm = _preprocess(
        x, edge_index, W1, b1, W2, b2, W3, b3)
    nc = _build(d_qg, off, ncol_pad)
    res = run_bass_kernel_spmd(nc, in_maps, core_ids=list(range(NCORES)))
    blocks = [res.results[k]["out"][:NBLK] for k in range(NCORES)]
    outp = np.concatenate(blocks, axis=0)
    out = np.empty((N, COUT), np.float32)
    out[perm] = outp
    return out


# revision 4
# speedup vs baseline: 1.9385x; 1.9385x over previous
"""3-layer GCN forward (GCNConv x3 + log_softmax) on 8 Trainium2 cores.

Strategy (self-contained; shapes hardcoded for N=100000, Cin=Ch=128,
Cout=47, 8 cores):
  A_hat = D^-1/2 (A+I) D^-1/2 is fixed across layers, so per layer
      out = dinv_dst * segsum_dst( dinv_src * (H @ W) ) + b
  Host: permute nodes into 8 contiguous core blocks (degree-sorted within
  each block), build per-core padded gather grids: 98 groups of 128 output
  rows, each with d_g gather steps (shared loop structure across cores).
  Device (SPMD, one NEFF on cores 0-7):
    per layer: tiled GEMM (fp32) + dinv_src row scale -> local Z block,
    cast to bf16; AllGather Z (bf16) across the 8 cores into a shared DRAM
    replica; aggregation: per group, d_g indirect-DMA row gathers (128
    bf16 rows/instr, 256B each) accumulated on the tensor engine via
    bf16 identity-matmul into fp32 PSUM; then dinv_dst scale + bias +
    relu (or log_softmax on the last layer).
  The next layer's GEMM for group g is emitted right after group g's
  aggregation postproc, so the tensor/vector engines overlap the gather
  stream and the gpsimd queue only stalls for the AllGather itself.

z_full row space: node (core k, local r) lives at row k*12544 + r; rows
[12500, 12544) of each block are zero pads; ZROW (=12500) backs unused
grid slots.
"""
import numpy as np

NCORES = 8
N = 100000
NBLK = 12500
NPAD = 12544            # 98 * 128
NGRP = NPAD // 128      # 98
C = 128
COUT = 47
ZROW = NBLK             # a zero pad row (core 0 block)


def _preprocess(x, edge_index, W1, b1, W2, b2, W3, b3):
    x = np.asarray(x, np.float32)
    ei = np.asarray(edge_index)
    loop = np.arange(N, dtype=np.int64)
    src = np.concatenate([ei[0], loop]).astype(np.int64)
    dst = np.concatenate([ei[1], loop]).astype(np.int64)

    deg = np.bincount(dst, minlength=N).astype(np.float32)
    dinv = 1.0 / np.sqrt(np.maximum(deg, 1.0))

    # deal degree-ranked nodes round-robin across cores so all 8 cores'
    # group degree profiles align (minimizes cross-core max padding)
    rank = np.argsort(-deg, kind="stable")
    perm = np.empty(N, np.int64)
    for k in range(NCORES):
        perm[k * NBLK:(k + 1) * NBLK] = rank[k::NCORES]
    inv = np.empty(N, np.int64)
    inv[perm] = np.arange(N)

    srcp = inv[src]
    dstp = inv[dst]
    ksrc = srcp // NBLK
    srcg = ksrc * NPAD + (srcp - ksrc * NBLK)     # padded-global coords

    dinv_p = dinv[perm]

    ecore = dstp // NBLK
    rloc = dstp - ecore * NBLK
    order = np.lexsort((srcg, rloc, ecore))
    ecore, rloc, srcg_s = ecore[order], rloc[order], srcg[order]

    flat = ecore * NBLK + rloc                     # sorted
    cnt = np.bincount(flat, minlength=NCORES * NBLK)
    cnt_pad = np.zeros(NCORES * NPAD, np.int64)
    idx_all = (np.arange(NCORES * NBLK) // NBLK) * NPAD + \
        (np.arange(NCORES * NBLK) % NBLK)
    cnt_pad[idx_all] = cnt
    d_per = cnt_pad.reshape(NCORES, NGRP, 128).max(axis=2)
    d_g = np.maximum(d_per.max(axis=0), 1).astype(np.int64)
    col_off = np.concatenate([[0], np.cumsum(d_g)])
    n_steps = int(col_off[-1])

    tables = np.full((NCORES, 128, n_steps), ZROW, np.int32)
    starts = np.zeros(NCORES * NBLK + 1, np.int64)
    np.cumsum(cnt, out=starts[1:])
    pos = np.arange(len(order)) - starts[flat]
    grp = rloc // 128
    part = rloc % 128
    colidx = col_off[grp] + pos
    tables[ecore, part, colidx] = srcg_s.astype(np.int32)

    dinv_loc = np.zeros((NCORES, 128, NGRP), np.float32)
    dv = dinv_p.reshape(NCORES, NBLK)
    for k in range(NCORES):
        full = np.zeros(NPAD, np.float32)
        full[:NBLK] = dv[k]
        dinv_loc[k] = full.reshape(NGRP, 128).T

    xp = x[perm]
    xblk = np.zeros((NCORES, NPAD, C), np.float32)
    for k in range(NCORES):
        xblk[k, :NBLK] = xp[k * NBLK:(k + 1) * NBLK]

    Ws = [np.ascontiguousarray(W, np.float32) for W in (W1, W2, W3)]
    bb = [np.tile(np.asarray(b, np.float32)[None, :], (128, 1))
          for b in (b1, b2, b3)]

    in_maps = []
    for k in range(NCORES):
        in_maps.append({
            "xblk": np.ascontiguousarray(xblk[k]),
            "gidx": np.ascontiguousarray(tables[k]),
            "dinv": np.ascontiguousarray(dinv_loc[k]),
            "w1": Ws[0], "w2": Ws[1], "w3": Ws[2],
            "bb1": np.ascontiguousarray(bb[0]),
            "bb2": np.ascontiguousarray(bb[1]),
            "bb3": np.ascontiguousarray(bb[2]),
        })
    return in_maps, [int(v) for v in d_g], n_steps, perm


def _build(d_g, n_steps):
    from concourse import bacc, bass, mybir, tile
    from concourse.masks import make_identity
    f32 = mybir.dt.float32
    bf16 = mybir.dt.bfloat16
    i32 = mybir.dt.int32
    couts = [C, C, COUT]

    nc = bacc.Bacc("TRN2", target_bir_lowering=False, debug=False,
                   num_devices=NCORES)
    xblk = nc.dram_tensor("xblk", [NPAD, C], f32, kind="ExternalInput")
    gidx = nc.dram_tensor("gidx", [128, n_steps], i32, kind="ExternalInput")
    dinv = nc.dram_tensor("dinv", [128, NGRP], f32, kind="ExternalInput")
    w_in = [nc.dram_tensor(f"w{l+1}", [C, couts[l]], f32,
                           kind="ExternalInput") for l in range(3)]
    bb_in = [nc.dram_tensor(f"bb{l+1}", [128, couts[l]], f32,
                            kind="ExternalInput") for l in range(3)]
    out_d = nc.dram_tensor("out", [NPAD, COUT], f32, kind="ExternalOutput")

    zs = [nc.dram_tensor(f"zs{l}", [NPAD, couts[l]], bf16) for l in range(3)]
    zf = [nc.dram_tensor(f"zf{l}", [NCORES * NPAD, couts[l]], bf16,
                         addr_space="Shared") for l in range(3)]

    with tile.TileContext(nc) as tc:
        with tc.tile_pool(name="const", bufs=1) as cpool, \
             tc.tile_pool(name="hbuf", bufs=1) as hpool, \
             tc.tile_pool(name="gath", bufs=10) as gpool, \
             tc.tile_pool(name="work", bufs=4) as wpool, \
             tc.tile_pool(name="ps_t", bufs=2, space="PSUM") as ps_t, \
             tc.tile_pool(name="ps_z", bufs=2, space="PSUM") as ps_z, \
             tc.tile_pool(name="ps_g", bufs=2, space="PSUM") as ps_g:

            ident = cpool.tile([128, 128], f32)
            make_identity(nc, ident[:])
            identB = cpool.tile([128, 128], bf16, name="identB")
            make_identity(nc, identB[:])
            idx_sb = cpool.tile([128, n_steps], i32)
            nc.sync.dma_start(out=idx_sb[:], in_=gidx[:])
            dinv_sb = cpool.tile([128, NGRP], f32)
            nc.sync.dma_start(out=dinv_sb[:], in_=dinv[:])
            w_sb, bb_sb = [], []
            for l in range(3):
                w = cpool.tile([128, couts[l]], f32, name=f"w_sb{l}")
                nc.sync.dma_start(out=w[:], in_=w_in[l][:])
                w_sb.append(w)
                b = cpool.tile([128, couts[l]], f32, name=f"bb_sb{l}")
                nc.sync.dma_start(out=b[:], in_=bb_in[l][:])
                bb_sb.append(b)

            H = hpool.tile([128, NGRP * C], f32)

            def phase_a(lay, g):
                """GEMM for layer `lay`, group g: H (or x) @ W -> zs[lay]."""
                co = couts[lay]
                if lay == 0:
                    hin = wpool.tile([128, C], f32, name="hin")
                    nc.sync.dma_start(
                        out=hin[:], in_=xblk[g * 128:(g + 1) * 128, :])
                    hsrc = hin[:]
                else:
                    hsrc = H[:, g * C:(g + 1) * C]
                pst = ps_t.tile([128, 128], f32, name="pst")
                nc.tensor.transpose(out=pst[:], in_=hsrc, identity=ident[:])
                ht = wpool.tile([128, 128], f32, name="ht")
                nc.vector.tensor_copy(out=ht[:], in_=pst[:])
                psz = ps_z.tile([128, co], f32, name="psz")
                nc.tensor.matmul(out=psz[:], lhsT=ht[:], rhs=w_sb[lay][:],
                                 start=True, stop=True)
                zt = wpool.tile([128, C], f32, name="zt")
                nc.vector.tensor_scalar_mul(out=zt[:, :co], in0=psz[:],
                                            scalar1=dinv_sb[:, g:g + 1])
                ztb = wpool.tile([128, C], bf16, name="ztb")
                nc.vector.tensor_copy(out=ztb[:, :co], in_=zt[:, :co])
                nc.sync.dma_start(out=zs[lay][g * 128:(g + 1) * 128, :],
                                  in_=ztb[:, :co])

            for g in range(NGRP):
                phase_a(0, g)

            for lay in range(3):
                co = couts[lay]
                nc.gpsimd.collective_compute(
                    "AllGather", mybir.AluOpType.bypass,
                    replica_groups=[list(range(NCORES))],
                    ins=[zs[lay][:, :]], outs=[zf[lay][:, :]])

                s = 0
                for g in range(NGRP):
                    d = d_g[g]
                    nq = min(4, d)
                    psg = ps_g.tile([128, 4 * C], f32, name="psg")
                    nch = (d + 3) // 4
                    jj = 0
                    for ch in range(nch):
                        w = min(4, d - jj)
                        gs4 = gpool.tile([128, 4 * C], bf16, name="gs")
                        for q in range(w):
                            nc.gpsimd.indirect_dma_start(
                                out=gs4[:, q * C:q * C + co], out_offset=None,
                                in_=zf[lay][:, :],
                                in_offset=bass.IndirectOffsetOnAxis(
                                    ap=idx_sb[:, s:s + 1], axis=0))
                            s += 1
                        nc.tensor.matmul(out=psg[:, :w * C], lhsT=identB[:],
                                         rhs=gs4[:, :w * C],
                                         start=(ch == 0), stop=(ch == nch - 1))
                        jj += w
                    tmp = wpool.tile([128, C], f32, name="tmp")
                    nc.vector.tensor_copy(out=tmp[:, :co], in_=psg[:, :co])
                    for q in range(1, nq):
                        nc.vector.tensor_add(out=tmp[:, :co], in0=tmp[:, :co],
                                             in1=psg[:, q * C:q * C + co])
                    nc.vector.tensor_scalar_mul(out=tmp[:, :co], in0=tmp[:, :co],
                                                scalar1=dinv_sb[:, g:g + 1])
                    nc.vector.tensor_add(out=tmp[:, :co], in0=tmp[:, :co],
                                         in1=bb_sb[lay][:])
                    if lay < 2:
                        nc.vector.tensor_scalar_max(
                            out=H[:, g * C:(g + 1) * C], in0=tmp[:, :co],
                            scalar1=0.0)
                        phase_a(lay + 1, g)
                    else:
                        mx = wpool.tile([128, 1], f32, name="mx")
                        nc.vector.tensor_reduce(
                            out=mx[:], in_=tmp[:, :co],
                            axis=mybir.AxisListType.X, op=mybir.AluOpType.max)
                        nmx = wpool.tile([128, 1], f32, name="nmx")
                        nc.vector.tensor_scalar_mul(out=nmx[:], in0=mx[:],
                                                    scalar1=-1.0)
                        ex = wpool.tile([128, C], f32, name="ex")
                        ssum = wpool.tile([128, 1], f32, name="ssum")
                        nc.scalar.activation(
                            out=ex[:, :co], in_=tmp[:, :co],
                            func=mybir.ActivationFunctionType.Exp,
                            bias=nmx[:], scale=1.0, accum_out=ssum[:])
                        lse = wpool.tile([128, 1], f32, name="lse")
                        nc.scalar.activation(
                            out=lse[:], in_=ssum[:],
                            func=mybir.ActivationFunctionType.Ln)
                        tot = wpool.tile([128, 1], f32, name="tot")
                        nc.vector.tensor_add(out=tot[:], in0=lse[:], in1=mx[:])
                        ot = wpool.tile([128, COUT], f32, name="ot")
                        nc.vector.tensor_scalar_sub(out=ot[:], in0=tmp[:, :co],
                                                    scalar1=tot[:])
                        nc.sync.dma_start(
                            out=out_d[g * 128:(g + 1) * 128, :], in_=ot[:])

    nc.compile()
    return nc


def kernel(x, edge_index, W1, b1, W2, b2, W3, b3):
    from concourse.bass_utils import run_bass_kernel_spmd

    in_maps, d_g, n_steps, perm = _preprocess(
        x, edge_index, W1, b1, W2, b2, W3, b3)
    nc = _build(d_g, n_steps)
    res = run_bass_kernel_spmd(nc, in_maps, core_ids=list(range(NCORES)))
    blocks = [res.results[k]["out"][:NBLK] for k in range(NCORES)]
    outp = np.concatenate(blocks, axis=0)
    out = np.empty((N, COUT), np.float32)
    out[perm] = outp
    return out


# revision 6
# speedup vs baseline: 1.9411x; 1.0014x over previous
"""3-layer GCN forward (GCNConv x3 + log_softmax) on 8 Trainium2 cores.

Strategy (self-contained; shapes hardcoded for N=100000, Cin=Ch=128,
Cout=47, 8 cores):
  A_hat = D^-1/2 (A+I) D^-1/2 is fixed across layers, so per layer
      out = dinv_dst * segsum_dst( dinv_src * (H @ W) ) + b
  Host: permute nodes into 8 contiguous core blocks (degree-sorted within
  each block), build per-core padded gather grids: 98 groups of 128 output
  rows, each with d_g gather steps (shared loop structure across cores).
  Device (SPMD, one NEFF on cores 0-7):
    per layer: tiled GEMM (fp32) + dinv_src row scale -> local Z block,
    cast to bf16; AllGather Z (bf16) across the 8 cores into a shared DRAM
    replica; aggregation: per group, d_g indirect-DMA row gathers (128
    bf16 rows/instr, 256B each) accumulated on the tensor engine via
    bf16 identity-matmul into fp32 PSUM; then dinv_dst scale + bias +
    relu (or log_softmax on the last layer).
  The next layer's GEMM for group g is emitted right after group g's
  aggregation postproc, so the tensor/vector engines overlap the gather
  stream and the gpsimd queue only stalls for the AllGather itself.

z_full row space: node (core k, local r) lives at row k*12544 + r; rows
[12500, 12544) of each block are zero pads; ZROW (=12500) backs unused
grid slots.
"""
import numpy as np

NCORES = 8
N = 100000
NBLK = 12500
NPAD = 12544            # 98 * 128
NGRP = NPAD // 128      # 98
C = 128
COUT = 47
ZROW = NBLK             # a zero pad row (core 0 block)


def _preprocess(x, edge_index, W1, b1, W2, b2, W3, b3):
    x = np.asarray(x, np.float32)
    ei = np.asarray(edge_index)
    loop = np.arange(N, dtype=np.int64)
    src = np.concatenate([ei[0], loop]).astype(np.int64)
    dst = np.concatenate([ei[1], loop]).astype(np.int64)

    deg = np.bincount(dst, minlength=N).astype(np.float32)
    dinv = 1.0 / np.sqrt(np.maximum(deg, 1.0))

    # deal degree-ranked nodes round-robin across cores so all 8 cores'
    # group degree profiles align (minimizes cross-core max padding)
    rank = np.argsort(-deg, kind="stable")
    perm = np.empty(N, np.int64)
    for k in range(NCORES):
        perm[k * NBLK:(k + 1) * NBLK] = rank[k::NCORES]
    inv = np.empty(N, np.int64)
    inv[perm] = np.arange(N)

    srcp = inv[src]
    dstp = inv[dst]
    ksrc = srcp // NBLK
    srcg = ksrc * NPAD + (srcp - ksrc * NBLK)     # padded-global coords

    dinv_p = dinv[perm]

    ecore = dstp // NBLK
    rloc = dstp - ecore * NBLK
    order = np.lexsort((srcg, rloc, ecore))
    ecore, rloc, srcg_s = ecore[order], rloc[order], srcg[order]

    flat = ecore * NBLK + rloc                     # sorted
    cnt = np.bincount(flat, minlength=NCORES * NBLK)
    cnt_pad = np.zeros(NCORES * NPAD, np.int64)
    idx_all = (np.arange(NCORES * NBLK) // NBLK) * NPAD + \
        (np.arange(NCORES * NBLK) % NBLK)
    cnt_pad[idx_all] = cnt
    d_per = cnt_pad.reshape(NCORES, NGRP, 128).max(axis=2)
    d_g = np.maximum(d_per.max(axis=0), 1).astype(np.int64)
    col_off = np.concatenate([[0], np.cumsum(d_g)])
    n_steps = int(col_off[-1])

    tables = np.full((NCORES, 128, n_steps), ZROW, np.int32)
    starts = np.zeros(NCORES * NBLK + 1, np.int64)
    np.cumsum(cnt, out=starts[1:])
    pos = np.arange(len(order)) - starts[flat]
    grp = rloc // 128
    part = rloc % 128
    colidx = col_off[grp] + pos
    tables[ecore, part, colidx] = srcg_s.astype(np.int32)

    dinv_loc = np.zeros((NCORES, 128, NGRP), np.float32)
    dv = dinv_p.reshape(NCORES, NBLK)
    for k in range(NCORES):
        full = np.zeros(NPAD, np.float32)
        full[:NBLK] = dv[k]
        dinv_loc[k] = full.reshape(NGRP, 128).T

    xp = x[perm]
    xblk = np.zeros((NCORES, NPAD, C), np.float32)
    for k in range(NCORES):
        xblk[k, :NBLK] = xp[k * NBLK:(k + 1) * NBLK]

    Ws = [np.ascontiguousarray(W, np.float32) for W in (W1, W2, W3)]
    bb = [np.tile(np.asarray(b, np.float32)[None, :], (128, 1))
          for b in (b1, b2, b3)]

    in_maps = []
    for k in range(NCORES):
        in_maps.append({
            "xblk": np.ascontiguousarray(xblk[k]),
            "gidx": np.ascontiguousarray(tables[k]),
            "dinv": np.ascontiguousarray(dinv_loc[k]),
            "w1": Ws[0], "w2": Ws[1], "w3": Ws[2],
            "bb1": np.ascontiguousarray(bb[0]),
            "bb2": np.ascontiguousarray(bb[1]),
            "bb3": np.ascontiguousarray(bb[2]),
        })
    return in_maps, [int(v) for v in d_g], n_steps, perm


def _build(d_g, n_steps):
    from concourse import bacc, bass, mybir, tile
    from concourse.masks import make_identity
    f32 = mybir.dt.float32
    bf16 = mybir.dt.bfloat16
    i32 = mybir.dt.int32
    couts = [C, C, COUT]

    nc = bacc.Bacc("TRN2", target_bir_lowering=False, debug=False,
                   num_devices=NCORES)
    xblk = nc.dram_tensor("xblk", [NPAD, C], f32, kind="ExternalInput")
    gidx = nc.dram_tensor("gidx", [128, n_steps], i32, kind="ExternalInput")
    dinv = nc.dram_tensor("dinv", [128, NGRP], f32, kind="ExternalInput")
    w_in = [nc.dram_tensor(f"w{l+1}", [C, couts[l]], f32,
                           kind="ExternalInput") for l in range(3)]
    bb_in = [nc.dram_tensor(f"bb{l+1}", [128, couts[l]], f32,
                            kind="ExternalInput") for l in range(3)]
    out_d = nc.dram_tensor("out", [NPAD, COUT], f32, kind="ExternalOutput")

    zs = [nc.dram_tensor(f"zs{l}", [NPAD, couts[l]], bf16) for l in range(3)]
    zf = [nc.dram_tensor(f"zf{l}", [NCORES * NPAD, couts[l]], bf16,
                         addr_space="Shared") for l in range(3)]

    with tile.TileContext(nc) as tc:
        with tc.tile_pool(name="const", bufs=1) as cpool, \
             tc.tile_pool(name="hbuf", bufs=1) as hpool, \
             tc.tile_pool(name="gath", bufs=6) as gpool, \
             tc.tile_pool(name="work", bufs=4) as wpool, \
             tc.tile_pool(name="ps_t", bufs=2, space="PSUM") as ps_t, \
             tc.tile_pool(name="ps_z", bufs=2, space="PSUM") as ps_z, \
             tc.tile_pool(name="ps_g", bufs=2, space="PSUM") as ps_g:

            ident = cpool.tile([128, 128], f32)
            make_identity(nc, ident[:])
            identB = cpool.tile([128, 128], bf16, name="identB")
            make_identity(nc, identB[:])
            idx_sb = cpool.tile([128, n_steps], i32)
            nc.sync.dma_start(out=idx_sb[:], in_=gidx[:])
            dinv_sb = cpool.tile([128, NGRP], f32)
            nc.sync.dma_start(out=dinv_sb[:], in_=dinv[:])
            w_sb, bb_sb = [], []
            for l in range(3):
                w = cpool.tile([128, couts[l]], f32, name=f"w_sb{l}")
                nc.sync.dma_start(out=w[:], in_=w_in[l][:])
                w_sb.append(w)
                b = cpool.tile([128, couts[l]], f32, name=f"bb_sb{l}")
                nc.sync.dma_start(out=b[:], in_=bb_in[l][:])
                bb_sb.append(b)

            H = hpool.tile([128, NGRP * C], f32)

            def phase_a(lay, g):
                """GEMM for layer `lay`, group g: H (or x) @ W -> zs[lay]."""
                co = couts[lay]
                if lay == 0:
                    hin = wpool.tile([128, C], f32, name="hin")
                    nc.sync.dma_start(
                        out=hin[:], in_=xblk[g * 128:(g + 1) * 128, :])
                    hsrc = hin[:]
                else:
                    hsrc = H[:, g * C:(g + 1) * C]
                pst = ps_t.tile([128, 128], f32, name="pst")
                nc.tensor.transpose(out=pst[:], in_=hsrc, identity=ident[:])
                ht = wpool.tile([128, 128], f32, name="ht")
                nc.vector.tensor_copy(out=ht[:], in_=pst[:])
                psz = ps_z.tile([128, co], f32, name="psz")
                nc.tensor.matmul(out=psz[:], lhsT=ht[:], rhs=w_sb[lay][:],
                                 start=True, stop=True)
                zt = wpool.tile([128, C], f32, name="zt")
                nc.vector.tensor_scalar_mul(out=zt[:, :co], in0=psz[:],
                                            scalar1=dinv_sb[:, g:g + 1])
                ztb = wpool.tile([128, C], bf16, name="ztb")
                nc.vector.tensor_copy(out=ztb[:, :co], in_=zt[:, :co])
                nc.sync.dma_start(out=zs[lay][g * 128:(g + 1) * 128, :],
                                  in_=ztb[:, :co])

            for g in range(NGRP):
                phase_a(0, g)

            for lay in range(3):
                co = couts[lay]
                nc.gpsimd.collective_compute(
                    "AllGather", mybir.AluOpType.bypass,
                    replica_groups=[list(range(NCORES))],
                    ins=[zs[lay][:, :]], outs=[zf[lay][:, :]])

                s = 0
                for g in range(NGRP):
                    d = d_g[g]
                    nq = min(4, d)
                    psg = ps_g.tile([128, 4 * C], f32, name="psg")
                    nch = (d + 3) // 4
                    ntile = (d + 7) // 8
                    ch = 0
                    for tix in range(ntile):
                        tw = min(8, d - tix * 8)
                        gs8 = gpool.tile([128, 8 * C], bf16, name="gs")
                        for q in range(tw):
                            nc.gpsimd.indirect_dma_start(
                                out=gs8[:, q * C:q * C + co], out_offset=None,
                                in_=zf[lay][:, :],
                                in_offset=bass.IndirectOffsetOnAxis(
                                    ap=idx_sb[:, s:s + 1], axis=0))
                            s += 1
                        off = 0
                        while off < tw:
                            w = min(4, tw - off)
                            nc.tensor.matmul(
                                out=psg[:, :w * C], lhsT=identB[:],
                                rhs=gs8[:, off * C:(off + w) * C],
                                start=(ch == 0), stop=(ch == nch - 1))
                            ch += 1
                            off += w
                    tmp = wpool.tile([128, C], f32, name="tmp")
                    nc.vector.tensor_copy(out=tmp[:, :co], in_=psg[:, :co])
                    for q in range(1, nq):
                        nc.vector.tensor_add(out=tmp[:, :co], in0=tmp[:, :co],
                                             in1=psg[:, q * C:q * C + co])
                    nc.vector.tensor_scalar_mul(out=tmp[:, :co], in0=tmp[:, :co],
                                                scalar1=dinv_sb[:, g:g + 1])
                    nc.vector.tensor_add(out=tmp[:, :co], in0=tmp[:, :co],
                                         in1=bb_sb[lay][:])
                    if lay < 2:
                        nc.vector.tensor_scalar_max(
                            out=H[:, g * C:(g + 1) * C], in0=tmp[:, :co],
                            scalar1=0.0)
                        phase_a(lay + 1, g)
                    else:
                        mx = wpool.tile([128, 1], f32, name="mx")
                        nc.vector.tensor_reduce(
                            out=mx[:], in_=tmp[:, :co],
                            axis=mybir.AxisListType.X, op=mybir.AluOpType.max)
                        nmx = wpool.tile([128, 1], f32, name="nmx")
                        nc.vector.tensor_scalar_mul(out=nmx[:], in0=mx[:],
                                                    scalar1=-1.0)
                        ex = wpool.tile([128, C], f32, name="ex")
                        ssum = wpool.tile([128, 1], f32, name="ssum")
                        nc.scalar.activation(
                            out=ex[:, :co], in_=tmp[:, :co],
                            func=mybir.ActivationFunctionType.Exp,
                            bias=nmx[:], scale=1.0, accum_out=ssum[:])
                        lse = wpool.tile([128, 1], f32, name="lse")
                        nc.scalar.activation(
                            out=lse[:], in_=ssum[:],
                            func=mybir.ActivationFunctionType.Ln)
                        tot = wpool.tile([128, 1], f32, name="tot")
                        nc.vector.tensor_add(out=tot[:], in0=lse[:], in1=mx[:])
                        ot = wpool.tile([128, COUT], f32, name="ot")
                        nc.vector.tensor_scalar_sub(out=ot[:], in0=tmp[:, :co],
                                                    scalar1=tot[:])
                        nc.sync.dma_start(
                            out=out_d[g * 128:(g + 1) * 128, :], in_=ot[:])

    nc.compile()
    return nc


def kernel(x, edge_index, W1, b1, W2, b2, W3, b3):
    from concourse.bass_utils import run_bass_kernel_spmd

    in_maps, d_g, n_steps, perm = _preprocess(
        x, edge_index, W1, b1, W2, b2, W3, b3)
    nc = _build(d_g, n_steps)
    res = run_bass_kernel_spmd(nc, in_maps, core_ids=list(range(NCORES)))
    blocks = [res.results[k]["out"][:NBLK] for k in range(NCORES)]
    outp = np.concatenate(blocks, axis=0)
    out = np.empty((N, COUT), np.float32)
    out[perm] = outp
    return out


# revision 9
# speedup vs baseline: 2.0390x; 1.0504x over previous
"""3-layer GCN forward (GCNConv x3 + log_softmax) on 8 Trainium2 cores.

Strategy (self-contained; shapes hardcoded for N=100000, Cin=Ch=128,
Cout=47, 8 cores):
  A_hat = D^-1/2 (A+I) D^-1/2 is fixed across layers, so per layer
      out = dinv_dst * segsum_dst( dinv_src * (H @ W) ) + b
  Host: permute nodes into 8 contiguous core blocks (degree-sorted within
  each block), build per-core padded gather grids: 98 groups of 128 output
  rows, each with d_g gather steps (shared loop structure across cores).
  Device (SPMD, one NEFF on cores 0-7):
    per layer: tiled GEMM (fp32) + dinv_src row scale -> local Z block,
    cast to bf16; AllGather Z (bf16) across the 8 cores into a shared DRAM
    replica; aggregation: per group, d_g indirect-DMA row gathers (128
    bf16 rows/instr, 256B each) accumulated on the tensor engine via
    bf16 identity-matmul into fp32 PSUM; then dinv_dst scale + bias +
    relu (or log_softmax on the last layer).
  The next layer's GEMM for group g is emitted right after group g's
  aggregation postproc, so the tensor/vector engines overlap the gather
  stream and the gpsimd queue only stalls for the AllGather itself.

z_full row space: node (core k, local r) lives at row k*12544 + r; rows
[12500, 12544) of each block are zero pads; ZROW (=12500) backs unused
grid slots.
"""
import numpy as np

NCORES = 8
N = 100000
NBLK = 12500
NPAD = 12544            # 98 * 128
NGRP = NPAD // 128      # 98
C = 128
COUT = 47
ZROW = NBLK             # a zero pad row (core 0 block)


def _preprocess(x, edge_index, W1, b1, W2, b2, W3, b3):
    x = np.asarray(x, np.float32)
    ei = np.asarray(edge_index)
    # grid holds real edges only; self-loops are added on-chip from the
    # locally-computed dinv*z (ZT buffer), saving one gather slot per row
    src = ei[0].astype(np.int64)
    dst = ei[1].astype(np.int64)

    deg = (np.bincount(dst, minlength=N) + 1).astype(np.float32)  # + self
    dinv = 1.0 / np.sqrt(np.maximum(deg, 1.0))

    # deal degree-ranked nodes round-robin across cores so all 8 cores'
    # group degree profiles align (minimizes cross-core max padding)
    rank = np.argsort(-deg, kind="stable")
    perm = np.empty(N, np.int64)
    for k in range(NCORES):
        perm[k * NBLK:(k + 1) * NBLK] = rank[k::NCORES]
    inv = np.empty(N, np.int64)
    inv[perm] = np.arange(N)

    srcp = inv[src]
    dstp = inv[dst]
    ksrc = srcp // NBLK
    srcg = ksrc * NPAD + (srcp - ksrc * NBLK)     # padded-global coords

    dinv_p = dinv[perm]

    ecore = dstp // NBLK
    rloc = dstp - ecore * NBLK
    order = np.lexsort((srcg, rloc, ecore))
    ecore, rloc, srcg_s = ecore[order], rloc[order], srcg[order]

    flat = ecore * NBLK + rloc                     # sorted
    cnt = np.bincount(flat, minlength=NCORES * NBLK)
    cnt_pad = np.zeros(NCORES * NPAD, np.int64)
    idx_all = (np.arange(NCORES * NBLK) // NBLK) * NPAD + \
        (np.arange(NCORES * NBLK) % NBLK)
    cnt_pad[idx_all] = cnt
    d_per = cnt_pad.reshape(NCORES, NGRP, 128).max(axis=2)
    d_g = np.maximum(d_per.max(axis=0), 1).astype(np.int64)
    col_off = np.concatenate([[0], np.cumsum(d_g)])
    n_steps = int(col_off[-1])

    tables = np.full((NCORES, 128, n_steps), ZROW, np.int32)
    starts = np.zeros(NCORES * NBLK + 1, np.int64)
    np.cumsum(cnt, out=starts[1:])
    pos = np.arange(len(order)) - starts[flat]
    grp = rloc // 128
    part = rloc % 128
    colidx = col_off[grp] + pos
    tables[ecore, part, colidx] = srcg_s.astype(np.int32)

    dinv_loc = np.zeros((NCORES, 128, NGRP), np.float32)
    dv = dinv_p.reshape(NCORES, NBLK)
    for k in range(NCORES):
        full = np.zeros(NPAD, np.float32)
        full[:NBLK] = dv[k]
        dinv_loc[k] = full.reshape(NGRP, 128).T

    xp = x[perm]
    xblk = np.zeros((NCORES, NPAD, C), np.float32)
    for k in range(NCORES):
        xblk[k, :NBLK] = xp[k * NBLK:(k + 1) * NBLK]

    Ws = [np.ascontiguousarray(W, np.float32) for W in (W1, W2, W3)]
    bb = [np.tile(np.asarray(b, np.float32)[None, :], (128, 1))
          for b in (b1, b2, b3)]

    in_maps = []
    for k in range(NCORES):
        in_maps.append({
            "xblk": np.ascontiguousarray(xblk[k]),
            "gidx": np.ascontiguousarray(tables[k]),
            "dinv": np.ascontiguousarray(dinv_loc[k]),
            "w1": Ws[0], "w2": Ws[1], "w3": Ws[2],
            "bb1": np.ascontiguousarray(bb[0]),
            "bb2": np.ascontiguousarray(bb[1]),
            "bb3": np.ascontiguousarray(bb[2]),
        })
    return in_maps, [int(v) for v in d_g], n_steps, perm


def _build(d_g, n_steps):
    from concourse import bacc, bass, mybir, tile
    from concourse.masks import make_identity
    f32 = mybir.dt.float32
    bf16 = mybir.dt.bfloat16
    i32 = mybir.dt.int32
    couts = [C, C, COUT]

    nc = bacc.Bacc("TRN2", target_bir_lowering=False, debug=False,
                   num_devices=NCORES)
    xblk = nc.dram_tensor("xblk", [NPAD, C], f32, kind="ExternalInput")
    gidx = nc.dram_tensor("gidx", [128, n_steps], i32, kind="ExternalInput")
    dinv = nc.dram_tensor("dinv", [128, NGRP], f32, kind="ExternalInput")
    w_in = [nc.dram_tensor(f"w{l+1}", [C, couts[l]], f32,
                           kind="ExternalInput") for l in range(3)]
    bb_in = [nc.dram_tensor(f"bb{l+1}", [128, couts[l]], f32,
                            kind="ExternalInput") for l in range(3)]
    out_d = nc.dram_tensor("out", [NPAD, COUT], f32, kind="ExternalOutput")

    zs = [nc.dram_tensor(f"zs{l}", [NPAD, couts[l]], bf16) for l in range(3)]
    zf = [nc.dram_tensor(f"zf{l}", [NCORES * NPAD, couts[l]], bf16,
                         addr_space="Shared") for l in range(3)]

    with tile.TileContext(nc) as tc:
        with tc.tile_pool(name="const", bufs=1) as cpool, \
             tc.tile_pool(name="hbuf", bufs=1) as hpool, \
             tc.tile_pool(name="gath", bufs=6) as gpool, \
             tc.tile_pool(name="work", bufs=4) as wpool, \
             tc.tile_pool(name="ps_t", bufs=2, space="PSUM") as ps_t, \
             tc.tile_pool(name="ps_z", bufs=2, space="PSUM") as ps_z, \
             tc.tile_pool(name="ps_g", bufs=2, space="PSUM") as ps_g:

            ident = cpool.tile([128, 128], f32)
            make_identity(nc, ident[:])
            identB = cpool.tile([128, 128], bf16, name="identB")
            make_identity(nc, identB[:])
            idx_sb = cpool.tile([128, n_steps], i32)
            nc.sync.dma_start(out=idx_sb[:], in_=gidx[:])
            dinv_sb = cpool.tile([128, NGRP], f32)
            nc.sync.dma_start(out=dinv_sb[:], in_=dinv[:])
            w_sb, bb_sb = [], []
            for l in range(3):
                w = cpool.tile([128, couts[l]], f32, name=f"w_sb{l}")
                nc.sync.dma_start(out=w[:], in_=w_in[l][:])
                w_sb.append(w)
                b = cpool.tile([128, couts[l]], f32, name=f"bb_sb{l}")
                nc.sync.dma_start(out=b[:], in_=bb_in[l][:])
                bb_sb.append(b)

            H = hpool.tile([128, NGRP * C], f32)
            ZT = hpool.tile([128, NGRP * C], f32, name="ZT")

            def phase_a(lay, g):
                """GEMM for layer `lay`, group g: H (or x) @ W -> zs[lay]."""
                co = couts[lay]
                if lay == 0:
                    hin = wpool.tile([128, C], f32, name="hin")
                    nc.sync.dma_start(
                        out=hin[:], in_=xblk[g * 128:(g + 1) * 128, :])
                    hsrc = hin[:]
                else:
                    hsrc = H[:, g * C:(g + 1) * C]
                pst = ps_t.tile([128, 128], f32, name="pst")
                nc.tensor.transpose(out=pst[:], in_=hsrc, identity=ident[:])
                ht = wpool.tile([128, 128], f32, name="ht")
                nc.vector.tensor_copy(out=ht[:], in_=pst[:])
                psz = ps_z.tile([128, co], f32, name="psz")
                nc.tensor.matmul(out=psz[:], lhsT=ht[:], rhs=w_sb[lay][:],
                                 start=True, stop=True)
                nc.vector.tensor_scalar_mul(
                    out=ZT[:, g * C:g * C + co], in0=psz[:],
                    scalar1=dinv_sb[:, g:g + 1])
                ztb = wpool.tile([128, C], bf16, name="ztb")
                nc.vector.tensor_copy(out=ztb[:, :co],
                                      in_=ZT[:, g * C:g * C + co])
                nc.sync.dma_start(out=zs[lay][g * 128:(g + 1) * 128, :],
                                  in_=ztb[:, :co])

            for g in range(NGRP):
                phase_a(0, g)

            for lay in range(3):
                co = couts[lay]
                nc.gpsimd.collective_compute(
                    "AllGather", mybir.AluOpType.bypass,
                    replica_groups=[list(range(NCORES))],
                    ins=[zs[lay][:, :]], outs=[zf[lay][:, :]])

                s = 0
                for g in range(NGRP):
                    d = d_g[g]
                    nq = min(4, d)
                    psg = ps_g.tile([128, 4 * C], f32, name="psg")
                    nch = (d + 3) // 4
                    ntile = (d + 7) // 8
                    ch = 0
                    for tix in range(ntile):
                        tw = min(8, d - tix * 8)
                        gs8 = gpool.tile([128, 8 * C], bf16, name="gs")
                        for q in range(tw):
                            nc.gpsimd.indirect_dma_start(
                                out=gs8[:, q * C:q * C + co], out_offset=None,
                                in_=zf[lay][:, :],
                                in_offset=bass.IndirectOffsetOnAxis(
                                    ap=idx_sb[:, s:s + 1], axis=0))
                            s += 1
                        off = 0
                        while off < tw:
                            w = min(4, tw - off)
                            nc.tensor.matmul(
                                out=psg[:, :w * C], lhsT=identB[:],
                                rhs=gs8[:, off * C:(off + w) * C],
                                start=(ch == 0), stop=(ch == nch - 1))
                            ch += 1
                            off += w
                    tmp = wpool.tile([128, C], f32, name="tmp")
                    nc.vector.tensor_copy(out=tmp[:, :co], in_=psg[:, :co])
                    for q in range(1, nq):
                        nc.vector.tensor_add(out=tmp[:, :co], in0=tmp[:, :co],
                                             in1=psg[:, q * C:q * C + co])
                    # self-loop term: dinv_self * z_self computed in phase A
                    nc.vector.tensor_add(out=tmp[:, :co], in0=tmp[:, :co],
                                         in1=ZT[:, g * C:g * C + co])
                    nc.vector.tensor_scalar_mul(out=tmp[:, :co], in0=tmp[:, :co],
                                                scalar1=dinv_sb[:, g:g + 1])
                    nc.vector.tensor_add(out=tmp[:, :co], in0=tmp[:, :co],
                                         in1=bb_sb[lay][:])
                    if lay < 2:
                        nc.vector.tensor_scalar_max(
                            out=H[:, g * C:(g + 1) * C], in0=tmp[:, :co],
                            scalar1=0.0)
                        phase_a(lay + 1, g)
                    else:
                        mx = wpool.tile([128, 1], f32, name="mx")
                        nc.vector.tensor_reduce(
                            out=mx[:], in_=tmp[:, :co],
                            axis=mybir.AxisListType.X, op=mybir.AluOpType.max)
                        nmx = wpool.tile([128, 1], f32, name="nmx")
                        nc.vector.tensor_scalar_mul(out=nmx[:], in0=mx[:],
                                                    scalar1=-1.0)
                        ex = wpool.tile([128, C], f32, name="ex")
                        ssum = wpool.tile([128, 1], f32, name="ssum")
                        nc.scalar.activation(
                            out=ex[:, :co], in_=tmp[:, :co],
                            func=mybir.ActivationFunctionType.Exp,
                            bias=nmx[:], scale=1.0, accum_out=ssum[:])
                        lse = wpool.tile([128, 1], f32, name="lse")
                        nc.scalar.activation(
                            out=lse[:], in_=ssum[:],
                            func=mybir.ActivationFunctionType.Ln)
                        tot = wpool.tile([128, 1], f32, name="tot")
                        nc.vector.tensor_add(out=tot[:], in0=lse[:], in1=mx[:])
                        ot = wpool.tile([128, COUT], f32, name="ot")
                        nc.vector.tensor_scalar_sub(out=ot[:], in0=tmp[:, :co],
                                                    scalar1=tot[:])
                        nc.sync.dma_start(
                            out=out_d[g * 128:(g + 1) * 128, :], in_=ot[:])

    nc.compile()
    return nc


def kernel(x, edge_index, W1, b1, W2, b2, W3, b3):
    from concourse.bass_utils import run_bass_kernel_spmd

    in_maps, d_g, n_steps, perm = _preprocess(
        x, edge_index, W1, b1, W2, b2, W3, b3)
    nc = _build(d_g, n_steps)
    res = run_bass_kernel_spmd(nc, in_maps, core_ids=list(range(NCORES)))
    blocks = [res.results[k]["out"][:NBLK] for k in range(NCORES)]
    outp = np.concatenate(blocks, axis=0)
    out = np.empty((N, COUT), np.float32)
    out[perm] = outp
    return out


# revision 16
# speedup vs baseline: 2.0475x; 1.0042x over previous
"""3-layer GCN forward (GCNConv x3 + log_softmax) on 8 Trainium2 cores.

Strategy (self-contained; shapes hardcoded for N=100000, Cin=Ch=128,
Cout=47, 8 cores):
  A_hat = D^-1/2 (A+I) D^-1/2 is fixed across layers, so per layer
      out = dinv_dst * segsum_dst( dinv_src * (H @ W) ) + b
  Host: permute nodes into 8 contiguous core blocks (degree-sorted within
  each block), build per-core padded gather grids: 98 groups of 128 output
  rows, each with d_g gather steps (shared loop structure across cores).
  Device (SPMD, one NEFF on cores 0-7):
    per layer: tiled GEMM (fp32) + dinv_src row scale -> local Z block,
    cast to bf16; AllGather Z (bf16) across the 8 cores into a shared DRAM
    replica; aggregation: per group, d_g indirect-DMA row gathers (128
    bf16 rows/instr, 256B each) accumulated on the tensor engine via
    bf16 identity-matmul into fp32 PSUM; then dinv_dst scale + bias +
    relu (or log_softmax on the last layer).
  The next layer's GEMM for group g is emitted right after group g's
  aggregation postproc, so the tensor/vector engines overlap the gather
  stream and the gpsimd queue only stalls for the AllGather itself.

z_full row space: node (core k, local r) lives at row k*12544 + r; rows
[12500, 12544) of each block are zero pads; ZROW (=12500) backs unused
grid slots.
"""
import numpy as np

NCORES = 8
N = 100000
NBLK = 12500
NPAD = 12544            # 98 * 128
NGRP = NPAD // 128      # 98
C = 128
COUT = 47
HCH = NPAD // 2         # 6272 rows per AllGather half-chunk (49 groups)
# zero pad row in chunk-major zf space: core 0 local row 12500 ->
# chunk 1, ZROW = 1*(8*HCH) + 0*HCH + (12500 - HCH)
ZROW = NCORES * HCH + (NBLK - HCH)


def _preprocess(x, edge_index, W1, b1, W2, b2, W3, b3):
    x = np.asarray(x, np.float32)
    ei = np.asarray(edge_index)
    # grid holds real edges only; self-loops are added on-chip from the
    # locally-computed dinv*z (ZT buffer), saving one gather slot per row
    src = ei[0].astype(np.int64)
    dst = ei[1].astype(np.int64)

    deg = (np.bincount(dst, minlength=N) + 1).astype(np.float32)  # + self
    dinv = 1.0 / np.sqrt(np.maximum(deg, 1.0))

    # deal degree-ranked nodes round-robin across cores so all 8 cores'
    # group degree profiles align (minimizes cross-core max padding)
    rank = np.argsort(-deg, kind="stable")
    perm = np.empty(N, np.int64)
    for k in range(NCORES):
        perm[k * NBLK:(k + 1) * NBLK] = rank[k::NCORES]
    inv = np.empty(N, np.int64)
    inv[perm] = np.arange(N)

    srcp = inv[src]
    dstp = inv[dst]
    ksrc = srcp // NBLK
    rsrc = srcp - ksrc * NBLK
    # chunk-major zf numbering (chunk = half core block of HCH rows) so the
    # AllGather can run as two halves overlapped with the gather stream:
    # zf row = c*(8*HCH) + k*HCH + (r % HCH), c = r // HCH
    csrc = rsrc // HCH
    srcg = csrc * (NCORES * HCH) + ksrc * HCH + (rsrc - csrc * HCH)

    dinv_p = dinv[perm]

    ecore = dstp // NBLK
    rloc = dstp - ecore * NBLK
    order = np.lexsort((srcg, rloc, ecore))
    ecore, rloc, srcg_s = ecore[order], rloc[order], srcg[order]

    flat = ecore * NBLK + rloc                     # sorted
    cnt = np.bincount(flat, minlength=NCORES * NBLK)
    cnt_pad = np.zeros(NCORES * NPAD, np.int64)
    idx_all = (np.arange(NCORES * NBLK) // NBLK) * NPAD + \
        (np.arange(NCORES * NBLK) % NBLK)
    cnt_pad[idx_all] = cnt
    d_per = cnt_pad.reshape(NCORES, NGRP, 128).max(axis=2)
    d_g = np.maximum(d_per.max(axis=0), 1).astype(np.int64)
    col_off = np.concatenate([[0], np.cumsum(d_g)])
    n_steps = int(col_off[-1])

    tables = np.full((NCORES, 128, n_steps), ZROW, np.int32)
    starts = np.zeros(NCORES * NBLK + 1, np.int64)
    np.cumsum(cnt, out=starts[1:])
    pos = np.arange(len(order)) - starts[flat]
    grp = rloc // 128
    part = rloc % 128
    colidx = col_off[grp] + pos
    tables[ecore, part, colidx] = srcg_s.astype(np.int32)

    dinv_loc = np.zeros((NCORES, 128, NGRP), np.float32)
    dv = dinv_p.reshape(NCORES, NBLK)
    for k in range(NCORES):
        full = np.zeros(NPAD, np.float32)
        full[:NBLK] = dv[k]
        dinv_loc[k] = full.reshape(NGRP, 128).T

    xp = x[perm]
    xblk = np.zeros((NCORES, C, NPAD), np.float32)   # pre-transposed
    for k in range(NCORES):
        xblk[k, :, :NBLK] = xp[k * NBLK:(k + 1) * NBLK].T

    Ws = [np.ascontiguousarray(W, np.float32) for W in (W1, W2, W3)]
    bb = [np.tile(np.asarray(b, np.float32)[None, :], (128, 1))
          for b in (b1, b2, b3)]

    in_maps = []
    for k in range(NCORES):
        in_maps.append({
            "xblk": np.ascontiguousarray(xblk[k]),
            "gidx": np.ascontiguousarray(tables[k]),
            "dinv": np.ascontiguousarray(dinv_loc[k]),
            "w1": Ws[0], "w2": Ws[1], "w3": Ws[2],
            "bb1": np.ascontiguousarray(bb[0]),
            "bb2": np.ascontiguousarray(bb[1]),
            "bb3": np.ascontiguousarray(bb[2]),
        })
    return in_maps, [int(v) for v in d_g], n_steps, perm


def _build(d_g, n_steps):
    from concourse import bacc, bass, mybir, tile
    from concourse.masks import make_identity
    f32 = mybir.dt.float32
    bf16 = mybir.dt.bfloat16
    i32 = mybir.dt.int32
    couts = [C, C, COUT]

    nc = bacc.Bacc("TRN2", target_bir_lowering=False, debug=False,
                   num_devices=NCORES)
    xblk = nc.dram_tensor("xblk", [C, NPAD], f32, kind="ExternalInput")
    gidx = nc.dram_tensor("gidx", [128, n_steps], i32, kind="ExternalInput")
    dinv = nc.dram_tensor("dinv", [128, NGRP], f32, kind="ExternalInput")
    w_in = [nc.dram_tensor(f"w{l+1}", [C, couts[l]], f32,
                           kind="ExternalInput") for l in range(3)]
    bb_in = [nc.dram_tensor(f"bb{l+1}", [128, couts[l]], f32,
                            kind="ExternalInput") for l in range(3)]
    out_d = nc.dram_tensor("out", [NPAD, COUT], f32, kind="ExternalOutput")

    zs = [nc.dram_tensor(f"zs{l}", [NPAD, couts[l]], bf16) for l in range(3)]
    zf = [nc.dram_tensor(f"zf{l}", [NCORES * NPAD, couts[l]], bf16,
                         addr_space="Shared") for l in range(3)]

    with tile.TileContext(nc) as tc:
        with tc.tile_pool(name="const", bufs=1) as cpool, \
             tc.tile_pool(name="hbuf", bufs=1) as hpool, \
             tc.tile_pool(name="gath", bufs=6) as gpool, \
             tc.tile_pool(name="work", bufs=4) as wpool, \
             tc.tile_pool(name="ps_t", bufs=2, space="PSUM") as ps_t, \
             tc.tile_pool(name="ps_z", bufs=2, space="PSUM") as ps_z, \
             tc.tile_pool(name="ps_g", bufs=2, space="PSUM") as ps_g:

            ident = cpool.tile([128, 128], f32)
            make_identity(nc, ident[:])
            identB = cpool.tile([128, 128], bf16, name="identB")
            make_identity(nc, identB[:])
            idx_sb = cpool.tile([128, n_steps], i32)
            nc.sync.dma_start(out=idx_sb[:], in_=gidx[:])
            dinv_sb = cpool.tile([128, NGRP], f32)
            nc.sync.dma_start(out=dinv_sb[:], in_=dinv[:])
            w_sb, bb_sb = [], []
            for l in range(3):
                w = cpool.tile([128, couts[l]], f32, name=f"w_sb{l}")
                nc.sync.dma_start(out=w[:], in_=w_in[l][:])
                w_sb.append(w)
                b = cpool.tile([128, couts[l]], f32, name=f"bb_sb{l}")
                nc.sync.dma_start(out=b[:], in_=bb_in[l][:])
                bb_sb.append(b)

            H = hpool.tile([128, NGRP * C], f32)
            ZT = hpool.tile([128, NGRP * C], f32, name="ZT")

            def phase_a(lay, g):
                """GEMM for layer `lay`, group g: H (or x) @ W -> zs[lay]."""
                co = couts[lay]
                if lay == 0:
                    # x arrives pre-transposed: load lhsT directly, no
                    # on-chip transpose round trip
                    ht = wpool.tile([128, 128], f32, name="ht")
                    nc.sync.dma_start(
                        out=ht[:], in_=xblk[:, g * 128:(g + 1) * 128])
                else:
                    pst = ps_t.tile([128, 128], f32, name="pst")
                    nc.tensor.transpose(out=pst[:], in_=H[:, g * C:(g + 1) * C],
                                        identity=ident[:])
                    ht = wpool.tile([128, 128], f32, name="ht")
                    nc.vector.tensor_copy(out=ht[:], in_=pst[:])
                psz = ps_z.tile([128, co], f32, name="psz")
                nc.tensor.matmul(out=psz[:], lhsT=ht[:], rhs=w_sb[lay][:],
                                 start=True, stop=True)
                nc.vector.tensor_scalar_mul(
                    out=ZT[:, g * C:g * C + co], in0=psz[:],
                    scalar1=dinv_sb[:, g:g + 1])
                ztb = wpool.tile([128, C], bf16, name="ztb")
                nc.vector.tensor_copy(out=ztb[:, :co],
                                      in_=ZT[:, g * C:g * C + co])
                nc.sync.dma_start(out=zs[lay][g * 128:(g + 1) * 128, :],
                                  in_=ztb[:, :co])

            def ag_half(l, c):
                """AllGather one half-chunk of zs[l] into chunk-major zf[l]."""
                nc.gpsimd.collective_compute(
                    "AllGather", mybir.AluOpType.bypass,
                    replica_groups=[list(range(NCORES))],
                    ins=[zs[l][c * HCH:(c + 1) * HCH, :]],
                    outs=[zf[l][c * NCORES * HCH:(c + 1) * NCORES * HCH, :]])

            for g in range(NGRP):
                phase_a(0, g)
                if g == NGRP // 2 - 1:
                    ag_half(0, 0)
            ag_half(0, 1)

            for lay in range(3):
                co = couts[lay]
                s = 0
                for g in range(NGRP):
                    d = d_g[g]
                    nq = min(4, d)
                    psg = ps_g.tile([128, 4 * C], f32, name="psg")
                    nch = (d + 3) // 4
                    ntile = (d + 7) // 8
                    ch = 0
                    for tix in range(ntile):
                        tw = min(8, d - tix * 8)
                        gs8 = gpool.tile([128, 8 * C], bf16, name="gs")
                        for q in range(tw):
                            nc.gpsimd.indirect_dma_start(
                                out=gs8[:, q * C:q * C + co], out_offset=None,
                                in_=zf[lay][:, :],
                                in_offset=bass.IndirectOffsetOnAxis(
                                    ap=idx_sb[:, s:s + 1], axis=0))
                            s += 1
                        off = 0
                        while off < tw:
                            w = min(4, tw - off)
                            nc.tensor.matmul(
                                out=psg[:, :w * C], lhsT=identB[:],
                                rhs=gs8[:, off * C:(off + w) * C],
                                start=(ch == 0), stop=(ch == nch - 1))
                            ch += 1
                            off += w
                    tmp = wpool.tile([128, C], f32, name="tmp")
                    nc.vector.tensor_copy(out=tmp[:, :co], in_=psg[:, :co])
                    for q in range(1, nq):
                        nc.vector.tensor_add(out=tmp[:, :co], in0=tmp[:, :co],
                                             in1=psg[:, q * C:q * C + co])
                    # self-loop term: dinv_self * z_self computed in phase A
                    nc.vector.tensor_add(out=tmp[:, :co], in0=tmp[:, :co],
                                         in1=ZT[:, g * C:g * C + co])
                    nc.vector.tensor_scalar_mul(out=tmp[:, :co], in0=tmp[:, :co],
                                                scalar1=dinv_sb[:, g:g + 1])
                    nc.vector.tensor_add(out=tmp[:, :co], in0=tmp[:, :co],
                                         in1=bb_sb[lay][:])
                    if lay < 2:
                        nc.vector.tensor_scalar_max(
                            out=H[:, g * C:(g + 1) * C], in0=tmp[:, :co],
                            scalar1=0.0)
                        phase_a(lay + 1, g)
                        if g == NGRP // 2 - 1:
                            ag_half(lay + 1, 0)
                        elif g == NGRP - 1:
                            ag_half(lay + 1, 1)
                    else:
                        mx = wpool.tile([128, 1], f32, name="mx")
                        nc.vector.tensor_reduce(
                            out=mx[:], in_=tmp[:, :co],
                            axis=mybir.AxisListType.X, op=mybir.AluOpType.max)
                        nmx = wpool.tile([128, 1], f32, name="nmx")
                        nc.vector.tensor_scalar_mul(out=nmx[:], in0=mx[:],
                                                    scalar1=-1.0)
                        ex = wpool.tile([128, C], f32, name="ex")
                        ssum = wpool.tile([128, 1], f32, name="ssum")
                        nc.scalar.activation(
                            out=ex[:, :co], in_=tmp[:, :co],
                            func=mybir.ActivationFunctionType.Exp,
                            bias=nmx[:], scale=1.0, accum_out=ssum[:])
                        lse = wpool.tile([128, 1], f32, name="lse")
                        nc.scalar.activation(
                            out=lse[:], in_=ssum[:],
                            func=mybir.ActivationFunctionType.Ln)
                        tot = wpool.tile([128, 1], f32, name="tot")
                        nc.vector.tensor_add(out=tot[:], in0=lse[:], in1=mx[:])
                        ot = wpool.tile([128, COUT], f32, name="ot")
                        nc.vector.tensor_scalar_sub(out=ot[:], in0=tmp[:, :co],
                                                    scalar1=tot[:])
                        nc.sync.dma_start(
                            out=out_d[g * 128:(g + 1) * 128, :], in_=ot[:])

    nc.compile()
    return nc


def kernel(x, edge_index, W1, b1, W2, b2, W3, b3):
    from concourse.bass_utils import run_bass_kernel_spmd

    in_maps, d_g, n_steps, perm = _preprocess(
        x, edge_index, W1, b1, W2, b2, W3, b3)
    nc = _build(d_g, n_steps)
    res = run_bass_kernel_spmd(nc, in_maps, core_ids=list(range(NCORES)))
    blocks = [res.results[k]["out"][:NBLK] for k in range(NCORES)]
    outp = np.concatenate(blocks, axis=0)
    out = np.empty((N, COUT), np.float32)
    out[perm] = outp
    return out


# revision 17
# speedup vs baseline: 2.0698x; 1.0109x over previous
"""3-layer GCN forward (GCNConv x3 + log_softmax) on 8 Trainium2 cores.

Strategy (self-contained; shapes hardcoded for N=100000, Cin=Ch=128,
Cout=47, 8 cores):
  A_hat = D^-1/2 (A+I) D^-1/2 is fixed across layers, so per layer
      out = dinv_dst * segsum_dst( dinv_src * (H @ W) ) + b
  Host: permute nodes into 8 contiguous core blocks (degree-sorted within
  each block), build per-core padded gather grids: 98 groups of 128 output
  rows, each with d_g gather steps (shared loop structure across cores).
  Device (SPMD, one NEFF on cores 0-7):
    per layer: tiled GEMM (fp32) + dinv_src row scale -> local Z block,
    cast to bf16; AllGather Z (bf16) across the 8 cores into a shared DRAM
    replica; aggregation: per group, d_g indirect-DMA row gathers (128
    bf16 rows/instr, 256B each) accumulated on the tensor engine via
    bf16 identity-matmul into fp32 PSUM; then dinv_dst scale + bias +
    relu (or log_softmax on the last layer).
  The next layer's GEMM for group g is emitted right after group g's
  aggregation postproc, so the tensor/vector engines overlap the gather
  stream and the gpsimd queue only stalls for the AllGather itself.

z_full row space: node (core k, local r) lives at row k*12544 + r; rows
[12500, 12544) of each block are zero pads; ZROW (=12500) backs unused
grid slots.
"""
import numpy as np

NCORES = 8
N = 100000
NBLK = 12500
NPAD = 12544            # 98 * 128
NGRP = NPAD // 128      # 98
C = 128
COUT = 47
HCH = NPAD // 2         # 6272 rows per AllGather half-chunk (49 groups)
# zero pad row in chunk-major zf space: core 0 local row 12500 ->
# chunk 1, ZROW = 1*(8*HCH) + 0*HCH + (12500 - HCH)
ZROW = NCORES * HCH + (NBLK - HCH)


def _preprocess(x, edge_index, W1, b1, W2, b2, W3, b3):
    x = np.asarray(x, np.float32)
    ei = np.asarray(edge_index)
    # grid holds real edges only; self-loops are added on-chip from the
    # locally-computed dinv*z (ZT buffer), saving one gather slot per row
    src = ei[0].astype(np.int64)
    dst = ei[1].astype(np.int64)

    deg = (np.bincount(dst, minlength=N) + 1).astype(np.float32)  # + self
    dinv = 1.0 / np.sqrt(np.maximum(deg, 1.0))

    # deal degree-ranked nodes round-robin across cores so all 8 cores'
    # group degree profiles align (minimizes cross-core max padding)
    rank = np.argsort(-deg, kind="stable")
    perm = np.empty(N, np.int64)
    for k in range(NCORES):
        perm[k * NBLK:(k + 1) * NBLK] = rank[k::NCORES]
    inv = np.empty(N, np.int64)
    inv[perm] = np.arange(N)

    srcp = inv[src]
    dstp = inv[dst]
    ksrc = srcp // NBLK
    rsrc = srcp - ksrc * NBLK
    # chunk-major zf numbering (chunk = half core block of HCH rows) so the
    # AllGather can run as two halves overlapped with the gather stream:
    # zf row = c*(8*HCH) + k*HCH + (r % HCH), c = r // HCH
    csrc = rsrc // HCH
    srcg = csrc * (NCORES * HCH) + ksrc * HCH + (rsrc - csrc * HCH)

    dinv_p = dinv[perm]

    ecore = dstp // NBLK
    rloc = dstp - ecore * NBLK
    order = np.lexsort((srcg, rloc, ecore))
    ecore, rloc, srcg_s = ecore[order], rloc[order], srcg[order]

    flat = ecore * NBLK + rloc                     # sorted
    cnt = np.bincount(flat, minlength=NCORES * NBLK)
    cnt_pad = np.zeros(NCORES * NPAD, np.int64)
    idx_all = (np.arange(NCORES * NBLK) // NBLK) * NPAD + \
        (np.arange(NCORES * NBLK) % NBLK)
    cnt_pad[idx_all] = cnt
    d_per = cnt_pad.reshape(NCORES, NGRP, 128).max(axis=2)
    d_g = np.maximum(d_per.max(axis=0), 1).astype(np.int64)
    col_off = np.concatenate([[0], np.cumsum(d_g)])
    n_steps = int(col_off[-1])

    tables = np.full((NCORES, 128, n_steps), ZROW, np.int32)
    starts = np.zeros(NCORES * NBLK + 1, np.int64)
    np.cumsum(cnt, out=starts[1:])
    pos = np.arange(len(order)) - starts[flat]
    grp = rloc // 128
    part = rloc % 128
    colidx = col_off[grp] + pos
    tables[ecore, part, colidx] = srcg_s.astype(np.int32)

    dinv_loc = np.zeros((NCORES, 128, NGRP), np.float32)
    dv = dinv_p.reshape(NCORES, NBLK)
    for k in range(NCORES):
        full = np.zeros(NPAD, np.float32)
        full[:NBLK] = dv[k]
        dinv_loc[k] = full.reshape(NGRP, 128).T

    xp = x[perm]
    xblk = np.zeros((NCORES, C, NPAD), np.float32)   # pre-transposed
    for k in range(NCORES):
        xblk[k, :, :NBLK] = xp[k * NBLK:(k + 1) * NBLK].T

    Ws = [np.ascontiguousarray(W, np.float32) for W in (W1, W2, W3)]
    bb = [np.tile(np.asarray(b, np.float32)[None, :], (128, 1))
          for b in (b1, b2, b3)]

    in_maps = []
    for k in range(NCORES):
        in_maps.append({
            "xblk": np.ascontiguousarray(xblk[k]),
            "gidx": np.ascontiguousarray(tables[k]),
            "dinv": np.ascontiguousarray(dinv_loc[k]),
            "w1": Ws[0], "w2": Ws[1], "w3": Ws[2],
            "bb1": np.ascontiguousarray(bb[0]),
            "bb2": np.ascontiguousarray(bb[1]),
            "bb3": np.ascontiguousarray(bb[2]),
        })
    return in_maps, [int(v) for v in d_g], n_steps, perm


def _build(d_g, n_steps):
    from concourse import bacc, bass, mybir, tile
    from concourse.masks import make_identity
    f32 = mybir.dt.float32
    bf16 = mybir.dt.bfloat16
    i32 = mybir.dt.int32
    couts = [C, C, COUT]

    nc = bacc.Bacc("TRN2", target_bir_lowering=False, debug=False,
                   num_devices=NCORES)
    xblk = nc.dram_tensor("xblk", [C, NPAD], f32, kind="ExternalInput")
    gidx = nc.dram_tensor("gidx", [128, n_steps], i32, kind="ExternalInput")
    dinv = nc.dram_tensor("dinv", [128, NGRP], f32, kind="ExternalInput")
    w_in = [nc.dram_tensor(f"w{l+1}", [C, couts[l]], f32,
                           kind="ExternalInput") for l in range(3)]
    bb_in = [nc.dram_tensor(f"bb{l+1}", [128, couts[l]], f32,
                            kind="ExternalInput") for l in range(3)]
    out_d = nc.dram_tensor("out", [NPAD, COUT], f32, kind="ExternalOutput")

    zs = [nc.dram_tensor(f"zs{l}", [NPAD, couts[l]], bf16) for l in range(3)]
    zf = [nc.dram_tensor(f"zf{l}", [NCORES * NPAD, couts[l]], bf16,
                         addr_space="Shared") for l in range(3)]

    with tile.TileContext(nc) as tc:
        with tc.tile_pool(name="const", bufs=1) as cpool, \
             tc.tile_pool(name="hbuf", bufs=1) as hpool, \
             tc.tile_pool(name="gath", bufs=12) as gpool, \
             tc.tile_pool(name="work", bufs=4) as wpool, \
             tc.tile_pool(name="ps_t", bufs=2, space="PSUM") as ps_t, \
             tc.tile_pool(name="ps_z", bufs=2, space="PSUM") as ps_z, \
             tc.tile_pool(name="ps_g", bufs=2, space="PSUM") as ps_g:

            ident = cpool.tile([128, 128], f32)
            make_identity(nc, ident[:])
            identB = cpool.tile([128, 128], bf16, name="identB")
            make_identity(nc, identB[:])
            idx_sb = cpool.tile([128, n_steps], i32)
            nc.sync.dma_start(out=idx_sb[:], in_=gidx[:])
            dinv_sb = cpool.tile([128, NGRP], f32)
            nc.sync.dma_start(out=dinv_sb[:], in_=dinv[:])
            w_sb, bb_sb = [], []
            for l in range(3):
                w = cpool.tile([128, couts[l]], f32, name=f"w_sb{l}")
                nc.sync.dma_start(out=w[:], in_=w_in[l][:])
                w_sb.append(w)
                b = cpool.tile([128, couts[l]], f32, name=f"bb_sb{l}")
                nc.sync.dma_start(out=b[:], in_=bb_in[l][:])
                bb_sb.append(b)

            H = hpool.tile([128, NGRP * C], f32)
            ZT = hpool.tile([128, NGRP * C], f32, name="ZT")

            def phase_a(lay, g):
                """GEMM for layer `lay`, group g: H (or x) @ W -> zs[lay]."""
                co = couts[lay]
                if lay == 0:
                    # x arrives pre-transposed: load lhsT directly, no
                    # on-chip transpose round trip
                    ht = wpool.tile([128, 128], f32, name="ht")
                    nc.sync.dma_start(
                        out=ht[:], in_=xblk[:, g * 128:(g + 1) * 128])
                else:
                    pst = ps_t.tile([128, 128], f32, name="pst")
                    nc.tensor.transpose(out=pst[:], in_=H[:, g * C:(g + 1) * C],
                                        identity=ident[:])
                    ht = wpool.tile([128, 128], f32, name="ht")
                    nc.vector.tensor_copy(out=ht[:], in_=pst[:])
                psz = ps_z.tile([128, co], f32, name="psz")
                nc.tensor.matmul(out=psz[:], lhsT=ht[:], rhs=w_sb[lay][:],
                                 start=True, stop=True)
                nc.vector.tensor_scalar_mul(
                    out=ZT[:, g * C:g * C + co], in0=psz[:],
                    scalar1=dinv_sb[:, g:g + 1])
                ztb = wpool.tile([128, C], bf16, name="ztb")
                nc.vector.tensor_copy(out=ztb[:, :co],
                                      in_=ZT[:, g * C:g * C + co])
                nc.sync.dma_start(out=zs[lay][g * 128:(g + 1) * 128, :],
                                  in_=ztb[:, :co])

            def ag_half(l, c):
                """AllGather one half-chunk of zs[l] into chunk-major zf[l]."""
                nc.gpsimd.collective_compute(
                    "AllGather", mybir.AluOpType.bypass,
                    replica_groups=[list(range(NCORES))],
                    ins=[zs[l][c * HCH:(c + 1) * HCH, :]],
                    outs=[zf[l][c * NCORES * HCH:(c + 1) * NCORES * HCH, :]])

            for g in range(NGRP):
                phase_a(0, g)
                if g == NGRP // 2 - 1:
                    ag_half(0, 0)
            ag_half(0, 1)

            for lay in range(3):
                co = couts[lay]
                s = 0
                for g in range(NGRP):
                    d = d_g[g]
                    nq = min(4, d)
                    psg = ps_g.tile([128, 4 * C], f32, name="psg")
                    nch = (d + 3) // 4
                    ntile = (d + 7) // 8
                    ch = 0
                    for tix in range(ntile):
                        tw = min(8, d - tix * 8)
                        gs8 = gpool.tile([128, 8 * C], bf16, name="gs")
                        for q in range(tw):
                            nc.gpsimd.indirect_dma_start(
                                out=gs8[:, q * C:q * C + co], out_offset=None,
                                in_=zf[lay][:, :],
                                in_offset=bass.IndirectOffsetOnAxis(
                                    ap=idx_sb[:, s:s + 1], axis=0))
                            s += 1
                        off = 0
                        while off < tw:
                            w = min(4, tw - off)
                            nc.tensor.matmul(
                                out=psg[:, :w * C], lhsT=identB[:],
                                rhs=gs8[:, off * C:(off + w) * C],
                                start=(ch == 0), stop=(ch == nch - 1))
                            ch += 1
                            off += w
                    tmp = wpool.tile([128, C], f32, name="tmp")
                    nc.vector.tensor_copy(out=tmp[:, :co], in_=psg[:, :co])
                    for q in range(1, nq):
                        nc.vector.tensor_add(out=tmp[:, :co], in0=tmp[:, :co],
                                             in1=psg[:, q * C:q * C + co])
                    # self-loop term: dinv_self * z_self computed in phase A
                    nc.vector.tensor_add(out=tmp[:, :co], in0=tmp[:, :co],
                                         in1=ZT[:, g * C:g * C + co])
                    nc.vector.tensor_scalar_mul(out=tmp[:, :co], in0=tmp[:, :co],
                                                scalar1=dinv_sb[:, g:g + 1])
                    nc.vector.tensor_add(out=tmp[:, :co], in0=tmp[:, :co],
                                         in1=bb_sb[lay][:])
                    if lay < 2:
                        nc.vector.tensor_scalar_max(
                            out=H[:, g * C:(g + 1) * C], in0=tmp[:, :co],
                            scalar1=0.0)
                        phase_a(lay + 1, g)
                        if g == NGRP // 2 - 1:
                            ag_half(lay + 1, 0)
                        elif g == NGRP - 1:
                            ag_half(lay + 1, 1)
                    else:
                        mx = wpool.tile([128, 1], f32, name="mx")
                        nc.vector.tensor_reduce(
                            out=mx[:], in_=tmp[:, :co],
                            axis=mybir.AxisListType.X, op=mybir.AluOpType.max)
                        nmx = wpool.tile([128, 1], f32, name="nmx")
                        nc.vector.tensor_scalar_mul(out=nmx[:], in0=mx[:],
                                                    scalar1=-1.0)
                        ex = wpool.tile([128, C], f32, name="ex")
                        ssum = wpool.tile([128, 1], f32, name="ssum")
                        nc.scalar.activation(
                            out=ex[:, :co], in_=tmp[:, :co],
                            func=mybir.ActivationFunctionType.Exp,
                            bias=nmx[:], scale=1.0, accum_out=ssum[:])
                        lse = wpool.tile([128, 1], f32, name="lse")
                        nc.scalar.activation(
                            out=lse[:], in_=ssum[:],
                            func=mybir.ActivationFunctionType.Ln)
                        tot = wpool.tile([128, 1], f32, name="tot")
                        nc.vector.tensor_add(out=tot[:], in0=lse[:], in1=mx[:])
                        ot = wpool.tile([128, COUT], f32, name="ot")
                        nc.vector.tensor_scalar_sub(out=ot[:], in0=tmp[:, :co],
                                                    scalar1=tot[:])
                        nc.sync.dma_start(
                            out=out_d[g * 128:(g + 1) * 128, :], in_=ot[:])

    nc.compile()
    return nc


def kernel(x, edge_index, W1, b1, W2, b2, W3, b3):
    from concourse.bass_utils import run_bass_kernel_spmd

    in_maps, d_g, n_steps, perm = _preprocess(
        x, edge_index, W1, b1, W2, b2, W3, b3)
    nc = _build(d_g, n_steps)
    res = run_bass_kernel_spmd(nc, in_maps, core_ids=list(range(NCORES)))
    blocks = [res.results[k]["out"][:NBLK] for k in range(NCORES)]
    outp = np.concatenate(blocks, axis=0)
    out = np.empty((N, COUT), np.float32)
    out[perm] = outp
    return out


# revision 20
# speedup vs baseline: 2.0805x; 1.0052x over previous
"""3-layer GCN forward (GCNConv x3 + log_softmax) on 8 Trainium2 cores.

Strategy (self-contained; shapes hardcoded for N=100000, Cin=Ch=128,
Cout=47, 8 cores):
  A_hat = D^-1/2 (A+I) D^-1/2 is fixed across layers, so per layer
      out = dinv_dst * segsum_dst( dinv_src * (H @ W) ) + b
  Host: permute nodes into 8 contiguous core blocks (degree-sorted within
  each block), build per-core padded gather grids: 98 groups of 128 output
  rows, each with d_g gather steps (shared loop structure across cores).
  Device (SPMD, one NEFF on cores 0-7):
    per layer: tiled GEMM (fp32) + dinv_src row scale -> local Z block,
    cast to bf16; AllGather Z (bf16) across the 8 cores into a shared DRAM
    replica; aggregation: per group, d_g indirect-DMA row gathers (128
    bf16 rows/instr, 256B each) accumulated on the tensor engine via
    bf16 identity-matmul into fp32 PSUM; then dinv_dst scale + bias +
    relu (or log_softmax on the last layer).
  The next layer's GEMM for group g is emitted right after group g's
  aggregation postproc, so the tensor/vector engines overlap the gather
  stream and the gpsimd queue only stalls for the AllGather itself.

z_full row space: node (core k, local r) lives at row k*12544 + r; rows
[12500, 12544) of each block are zero pads; ZROW (=12500) backs unused
grid slots.
"""
import numpy as np

NCORES = 8
N = 100000
NBLK = 12500
NPAD = 12544            # 98 * 128
NGRP = NPAD // 128      # 98
C = 128
COUT = 47
NCHK = 7                # AllGather chunks per layer (14 groups each)
HCH = NPAD // NCHK      # 1792 rows per AllGather chunk
GCHK = NGRP // NCHK     # 14 groups per chunk
# zero pad row in chunk-major zf space: core 0 local row 12500
ZROW = (NBLK // HCH) * (NCORES * HCH) + (NBLK % HCH)


def _preprocess(x, edge_index, W1, b1, W2, b2, W3, b3):
    x = np.asarray(x, np.float32)
    ei = np.asarray(edge_index)
    # grid holds real edges only; self-loops are added on-chip from the
    # locally-computed dinv*z (ZT buffer), saving one gather slot per row
    src = ei[0].astype(np.int64)
    dst = ei[1].astype(np.int64)

    deg = (np.bincount(dst, minlength=N) + 1).astype(np.float32)  # + self
    dinv = 1.0 / np.sqrt(np.maximum(deg, 1.0))

    # deal degree-ranked nodes round-robin across cores so all 8 cores'
    # group degree profiles align (minimizes cross-core max padding)
    rank = np.argsort(-deg, kind="stable")
    perm = np.empty(N, np.int64)
    for k in range(NCORES):
        perm[k * NBLK:(k + 1) * NBLK] = rank[k::NCORES]
    inv = np.empty(N, np.int64)
    inv[perm] = np.arange(N)

    srcp = inv[src]
    dstp = inv[dst]
    ksrc = srcp // NBLK
    rsrc = srcp - ksrc * NBLK
    # chunk-major zf numbering (chunk = half core block of HCH rows) so the
    # AllGather can run as two halves overlapped with the gather stream:
    # zf row = c*(8*HCH) + k*HCH + (r % HCH), c = r // HCH
    csrc = rsrc // HCH
    srcg = csrc * (NCORES * HCH) + ksrc * HCH + (rsrc - csrc * HCH)

    dinv_p = dinv[perm]

    ecore = dstp // NBLK
    rloc = dstp - ecore * NBLK
    order = np.lexsort((srcg, rloc, ecore))
    ecore, rloc, srcg_s = ecore[order], rloc[order], srcg[order]

    flat = ecore * NBLK + rloc                     # sorted
    cnt = np.bincount(flat, minlength=NCORES * NBLK)
    cnt_pad = np.zeros(NCORES * NPAD, np.int64)
    idx_all = (np.arange(NCORES * NBLK) // NBLK) * NPAD + \
        (np.arange(NCORES * NBLK) % NBLK)
    cnt_pad[idx_all] = cnt
    d_per = cnt_pad.reshape(NCORES, NGRP, 128).max(axis=2)
    d_g = np.maximum(d_per.max(axis=0), 1).astype(np.int64)
    col_off = np.concatenate([[0], np.cumsum(d_g)])
    n_steps = int(col_off[-1])

    tables = np.full((NCORES, 128, n_steps), ZROW, np.int32)
    starts = np.zeros(NCORES * NBLK + 1, np.int64)
    np.cumsum(cnt, out=starts[1:])
    pos = np.arange(len(order)) - starts[flat]
    grp = rloc // 128
    part = rloc % 128
    colidx = col_off[grp] + pos
    tables[ecore, part, colidx] = srcg_s.astype(np.int32)

    dinv_loc = np.zeros((NCORES, 128, NGRP), np.float32)
    dv = dinv_p.reshape(NCORES, NBLK)
    for k in range(NCORES):
        full = np.zeros(NPAD, np.float32)
        full[:NBLK] = dv[k]
        dinv_loc[k] = full.reshape(NGRP, 128).T

    xp = x[perm]
    xblk = np.zeros((NCORES, C, NPAD), np.float32)   # pre-transposed
    for k in range(NCORES):
        xblk[k, :, :NBLK] = xp[k * NBLK:(k + 1) * NBLK].T

    Ws = [np.ascontiguousarray(W, np.float32) for W in (W1, W2, W3)]
    bb = [np.tile(np.asarray(b, np.float32)[None, :], (128, 1))
          for b in (b1, b2, b3)]

    in_maps = []
    for k in range(NCORES):
        in_maps.append({
            "xblk": np.ascontiguousarray(xblk[k]),
            "gidx": np.ascontiguousarray(tables[k]),
            "dinv": np.ascontiguousarray(dinv_loc[k]),
            "w1": Ws[0], "w2": Ws[1], "w3": Ws[2],
            "bb1": np.ascontiguousarray(bb[0]),
            "bb2": np.ascontiguousarray(bb[1]),
            "bb3": np.ascontiguousarray(bb[2]),
        })
    return in_maps, [int(v) for v in d_g], n_steps, perm


def _build(d_g, n_steps):
    from concourse import bacc, bass, mybir, tile
    from concourse.masks import make_identity
    f32 = mybir.dt.float32
    bf16 = mybir.dt.bfloat16
    i32 = mybir.dt.int32
    couts = [C, C, COUT]

    nc = bacc.Bacc("TRN2", target_bir_lowering=False, debug=False,
                   num_devices=NCORES)
    xblk = nc.dram_tensor("xblk", [C, NPAD], f32, kind="ExternalInput")
    gidx = nc.dram_tensor("gidx", [128, n_steps], i32, kind="ExternalInput")
    dinv = nc.dram_tensor("dinv", [128, NGRP], f32, kind="ExternalInput")
    w_in = [nc.dram_tensor(f"w{l+1}", [C, couts[l]], f32,
                           kind="ExternalInput") for l in range(3)]
    bb_in = [nc.dram_tensor(f"bb{l+1}", [128, couts[l]], f32,
                            kind="ExternalInput") for l in range(3)]
    out_d = nc.dram_tensor("out", [NPAD, COUT], f32, kind="ExternalOutput")

    zs = [nc.dram_tensor(f"zs{l}", [NPAD, couts[l]], bf16) for l in range(3)]
    zf = [nc.dram_tensor(f"zf{l}", [NCORES * NPAD, couts[l]], bf16,
                         addr_space="Shared") for l in range(3)]

    with tile.TileContext(nc) as tc:
        with tc.tile_pool(name="const", bufs=1) as cpool, \
             tc.tile_pool(name="hbuf", bufs=1) as hpool, \
             tc.tile_pool(name="gath", bufs=12) as gpool, \
             tc.tile_pool(name="work", bufs=4) as wpool, \
             tc.tile_pool(name="ps_t", bufs=2, space="PSUM") as ps_t, \
             tc.tile_pool(name="ps_z", bufs=2, space="PSUM") as ps_z, \
             tc.tile_pool(name="ps_g", bufs=2, space="PSUM") as ps_g:

            ident = cpool.tile([128, 128], f32)
            make_identity(nc, ident[:])
            identB = cpool.tile([128, 128], bf16, name="identB")
            make_identity(nc, identB[:])
            idx_sb = cpool.tile([128, n_steps], i32)
            nc.sync.dma_start(out=idx_sb[:], in_=gidx[:])
            dinv_sb = cpool.tile([128, NGRP], f32)
            nc.sync.dma_start(out=dinv_sb[:], in_=dinv[:])
            w_sb, bb_sb = [], []
            for l in range(3):
                w = cpool.tile([128, couts[l]], f32, name=f"w_sb{l}")
                nc.sync.dma_start(out=w[:], in_=w_in[l][:])
                w_sb.append(w)
                b = cpool.tile([128, couts[l]], f32, name=f"bb_sb{l}")
                nc.sync.dma_start(out=b[:], in_=bb_in[l][:])
                bb_sb.append(b)

            H = hpool.tile([128, NGRP * C], f32)
            ZT = hpool.tile([128, NGRP * C], f32, name="ZT")

            def phase_a(lay, g):
                """GEMM for layer `lay`, group g: H (or x) @ W -> zs[lay]."""
                co = couts[lay]
                if lay == 0:
                    # x arrives pre-transposed: load lhsT directly, no
                    # on-chip transpose round trip
                    ht = wpool.tile([128, 128], f32, name="ht")
                    nc.sync.dma_start(
                        out=ht[:], in_=xblk[:, g * 128:(g + 1) * 128])
                else:
                    pst = ps_t.tile([128, 128], f32, name="pst")
                    nc.tensor.transpose(out=pst[:], in_=H[:, g * C:(g + 1) * C],
                                        identity=ident[:])
                    ht = wpool.tile([128, 128], f32, name="ht")
                    nc.vector.tensor_copy(out=ht[:], in_=pst[:])
                psz = ps_z.tile([128, co], f32, name="psz")
                nc.tensor.matmul(out=psz[:], lhsT=ht[:], rhs=w_sb[lay][:],
                                 start=True, stop=True)
                nc.vector.tensor_scalar_mul(
                    out=ZT[:, g * C:g * C + co], in0=psz[:],
                    scalar1=dinv_sb[:, g:g + 1])
                ztb = wpool.tile([128, C], bf16, name="ztb")
                nc.vector.tensor_copy(out=ztb[:, :co],
                                      in_=ZT[:, g * C:g * C + co])
                nc.sync.dma_start(out=zs[lay][g * 128:(g + 1) * 128, :],
                                  in_=ztb[:, :co])

            def ag_half(l, c):
                """AllGather one half-chunk of zs[l] into chunk-major zf[l]."""
                nc.gpsimd.collective_compute(
                    "AllGather", mybir.AluOpType.bypass,
                    replica_groups=[list(range(NCORES))],
                    ins=[zs[l][c * HCH:(c + 1) * HCH, :]],
                    outs=[zf[l][c * NCORES * HCH:(c + 1) * NCORES * HCH, :]])

            for g in range(NGRP):
                phase_a(0, g)
                if (g + 1) % GCHK == 0:
                    ag_half(0, (g + 1) // GCHK - 1)

            for lay in range(3):
                co = couts[lay]
                s = 0
                for g in range(NGRP):
                    d = d_g[g]
                    nq = min(4, d)
                    psg = ps_g.tile([128, 4 * C], f32, name="psg")
                    nch = (d + 3) // 4
                    ntile = (d + 7) // 8
                    ch = 0
                    for tix in range(ntile):
                        tw = min(8, d - tix * 8)
                        gs8 = gpool.tile([128, 8 * C], bf16, name="gs")
                        for q in range(tw):
                            nc.gpsimd.indirect_dma_start(
                                out=gs8[:, q * C:q * C + co], out_offset=None,
                                in_=zf[lay][:, :],
                                in_offset=bass.IndirectOffsetOnAxis(
                                    ap=idx_sb[:, s:s + 1], axis=0))
                            s += 1
                        off = 0
                        while off < tw:
                            w = min(4, tw - off)
                            nc.tensor.matmul(
                                out=psg[:, :w * C], lhsT=identB[:],
                                rhs=gs8[:, off * C:(off + w) * C],
                                start=(ch == 0), stop=(ch == nch - 1))
                            ch += 1
                            off += w
                    tmp = wpool.tile([128, C], f32, name="tmp")
                    nc.vector.tensor_copy(out=tmp[:, :co], in_=psg[:, :co])
                    for q in range(1, nq):
                        nc.vector.tensor_add(out=tmp[:, :co], in0=tmp[:, :co],
                                             in1=psg[:, q * C:q * C + co])
                    # self-loop term: dinv_self * z_self computed in phase A
                    nc.vector.tensor_add(out=tmp[:, :co], in0=tmp[:, :co],
                                         in1=ZT[:, g * C:g * C + co])
                    nc.vector.tensor_scalar_mul(out=tmp[:, :co], in0=tmp[:, :co],
                                                scalar1=dinv_sb[:, g:g + 1])
                    nc.vector.tensor_add(out=tmp[:, :co], in0=tmp[:, :co],
                                         in1=bb_sb[lay][:])
                    if lay < 2:
                        nc.vector.tensor_scalar_max(
                            out=H[:, g * C:(g + 1) * C], in0=tmp[:, :co],
                            scalar1=0.0)
                        phase_a(lay + 1, g)
                        if (g + 1) % GCHK == 0:
                            ag_half(lay + 1, (g + 1) // GCHK - 1)
                    else:
                        mx = wpool.tile([128, 1], f32, name="mx")
                        nc.vector.tensor_reduce(
                            out=mx[:], in_=tmp[:, :co],
                            axis=mybir.AxisListType.X, op=mybir.AluOpType.max)
                        nmx = wpool.tile([128, 1], f32, name="nmx")
                        nc.vector.tensor_scalar_mul(out=nmx[:], in0=mx[:],
                                                    scalar1=-1.0)
                        ex = wpool.tile([128, C], f32, name="ex")
                        ssum = wpool.tile([128, 1], f32, name="ssum")
                        nc.scalar.activation(
                            out=ex[:, :co], in_=tmp[:, :co],
                            func=mybir.ActivationFunctionType.Exp,
                            bias=nmx[:], scale=1.0, accum_out=ssum[:])
                        lse = wpool.tile([128, 1], f32, name="lse")
                        nc.scalar.activation(
                            out=lse[:], in_=ssum[:],
                            func=mybir.ActivationFunctionType.Ln)
                        tot = wpool.tile([128, 1], f32, name="tot")
                        nc.vector.tensor_add(out=tot[:], in0=lse[:], in1=mx[:])
                        ot = wpool.tile([128, COUT], f32, name="ot")
                        nc.vector.tensor_scalar_sub(out=ot[:], in0=tmp[:, :co],
                                                    scalar1=tot[:])
                        nc.sync.dma_start(
                            out=out_d[g * 128:(g + 1) * 128, :], in_=ot[:])

    nc.compile()
    return nc


def kernel(x, edge_index, W1, b1, W2, b2, W3, b3):
    from concourse.bass_utils import run_bass_kernel_spmd

    in_maps, d_g, n_steps, perm = _preprocess(
        x, edge_index, W1, b1, W2, b2, W3, b3)
    nc = _build(d_g, n_steps)
    res = run_bass_kernel_spmd(nc, in_maps, core_ids=list(range(NCORES)))
    blocks = [res.results[k]["out"][:NBLK] for k in range(NCORES)]
    outp = np.concatenate(blocks, axis=0)
    out = np.empty((N, COUT), np.float32)
    out[perm] = outp
    return out
